# revision 11
# baseline (speedup 1.0000x reference)
"""Trainium2 Bass kernel for BoringFeedForwardMOE (E=8 experts, top-2, cap=2048).

Strategy: expert parallelism across 8 NeuronCores.
  - Router computed data-parallel (each core: logits for its 1024-token slice,
    f32 to match reference top-k decisions bit-for-bit), AllGather of logits.
  - Full routing (top-2 + softmax gates + capacity ranks) recomputed identically
    on every core from the gathered [8192, 8] logits; prefix sums over tokens
    via log-step shifted adds (free axis) + strictly-upper-triangular matmul
    (partition axis).
  - Dispatch: one batched dma_scatter_add builds buf (slot -> token id) in
    DRAM (unselected/over-capacity entries land in a dump row); the FFN input
    is then produced chunk-by-chunk with dma_gather(transpose=True), which
    reads each slot's bf16 x-row from HBM directly into [feature, slot]
    layout for the TensorEngine.
  - FFN in bf16, gelu + biases fused. Outputs are AllGathered in 4 chunks of
    512 slots so the collective overlaps the FFN of later chunks.
  - Combine: two batched dma_gathers fetch each own token's <=2 expert
    contributions (dropped ones hit a zeroed pad row), f32 gates, write out.

Token layout on device: t = p*64 + n (p: partition 0..127, n: 0..63).
Core c owns tokens [c*1024, (c+1)*1024) and expert e = c.
agout row for contribution (e, r): (r>>9)*4096 + e*512 + (r & 511).
"""
import sys

if "/opt/trn_rl_repo" not in sys.path:
    sys.path.insert(0, "/opt/trn_rl_repo")

import numpy as np
import ml_dtypes

import concourse.bass as bass
import concourse.bacc as bacc
import concourse.mybir as mybir
from concourse.tile import TileContext
from concourse.bass_utils import run_bass_kernel_spmd

F32 = mybir.dt.float32
BF16 = mybir.dt.bfloat16
I32 = mybir.dt.int32
AF = mybir.ActivationFunctionType
OP = mybir.AluOpType
AX = mybir.AxisListType

NCORES = 8
P = 128
NTOK = 8192          # B*T
C = 1024
HD = 4096
E = 8
CAP = 2048
NT = NTOK // P       # 64 token columns per partition
BIG = 1.0e6
TCH = 256            # FFN slot chunk (per PSUM group)
NCH = CAP // TCH     # 8 chunks
AGCH = 512           # slots per output-AllGather chunk
NAG = CAP // AGCH    # 4 collectives

_BUILT = None


def _r3(ap, e=E):
    return ap.rearrange("p (n e) -> p n e", e=e)


def build(debug=False):
    nc = bacc.Bacc()

    # ---- per-core parameters -------------------------------------------------
    xt = nc.declare_dram_parameter("xt", [C, 1024], F32, isOutput=False)
    xb = nc.declare_dram_parameter("xb", [NTOK, C], BF16, isOutput=False)
    wr = nc.declare_dram_parameter("wr", [P, 64], F32, isOutput=False)
    brr = nc.declare_dram_parameter("brr", [1, E], F32, isOutput=False)
    w1 = nc.declare_dram_parameter("w1", [C, HD], BF16, isOutput=False)
    b1a = nc.declare_dram_parameter("b1a", [P, HD // P], F32, isOutput=False)
    w2 = nc.declare_dram_parameter("w2", [HD, C], BF16, isOutput=False)
    b2r = nc.declare_dram_parameter("b2r", [1, C], F32, isOutput=False)
    tri = nc.declare_dram_parameter("tri", [P, P], F32, isOutput=False)
    binv = nc.declare_dram_parameter("binv", [P, NT * E], F32, isOutput=False)
    cmask = nc.declare_dram_parameter("cmask", [P, NT * E], F32, isOutput=False)
    ones1 = nc.declare_dram_parameter("ones1", [1, P], F32, isOutput=False)
    cidp = nc.declare_dram_parameter("cidp", [P, 1], I32, isOutput=False)
    tokid = nc.declare_dram_parameter("tokid", [P, NT], mybir.dt.int16,
                                      isOutput=False)
    out = nc.declare_dram_parameter("out", [1024, C], F32, isOutput=True)
    if debug:
        dbg_lg = nc.declare_dram_parameter("dbg_lg", [NTOK, E], F32, isOutput=True)
        dbg_rs = nc.declare_dram_parameter("dbg_rs", [1024, 32], F32, isOutput=True)
        dbg_bf = nc.declare_dram_parameter("dbg_bf", [CAP + P, P], F32, isOutput=True)
        dbg_ag = nc.declare_dram_parameter("dbg_ag", [E * CAP, C], F32, isOutput=True)

    # ---- internal DRAM -------------------------------------------------------
    lgin = nc.dram_tensor("lgin", [1024, E], F32)
    lgout = nc.dram_tensor("lgout", [NTOK, E], F32, addr_space="Shared")
    # buf_d row s (first int16) = token id at slot s of this expert; row CAP is
    # the dump row for unselected / over-capacity entries.
    buf_d = nc.dram_tensor("buf_d", [CAP + P, P], mybir.dt.int16)
    xg = nc.dram_tensor("xg", [CAP, C], BF16)
    didx_d = nc.dram_tensor("didx_d", [NTOK], mybir.dt.int16)
    cscr = nc.dram_tensor("cscr", [2, 1024], mybir.dt.int16)
    agin = nc.dram_tensor("agin", [CAP, C], BF16)
    # chunk k of agin ([AGCH, C] slots) -> agout rows [k*E*AGCH, (k+1)*E*AGCH);
    # rows [E*CAP, E*CAP+P) are a zeroed pad target for dropped contributions.
    agout = nc.dram_tensor("agout", [E * CAP + P, C], BF16, addr_space="Shared")
    rsc = nc.dram_tensor("rsc", [NTOK // 8, 32], F32)   # packed idx1/idx2/g1/g2

    rg = [list(range(NCORES))]

    with TileContext(nc) as tc:
        with tc.tile_pool(name="wpool", bufs=1) as wp:
            on1 = wp.tile([1, P], F32, tag="ones1")
            nc.sync.dma_start(out=on1[:], in_=ones1[:])
            cis = wp.tile([P, 1], I32, tag="cidp")
            nc.sync.dma_start(out=cis[:], in_=cidp[:])
            tks = wp.tile([P, NT], mybir.dt.int16, tag="tokid")
            nc.sync.dma_start(out=tks[:], in_=tokid[:])
            # zero-fill buf_d (scatter-add target) and the agout pad rows
            zi = wp.tile([P, P], mybir.dt.int16, tag="zi")
            nc.vector.memset(zi[:], 0)
            for i in range((CAP + P) // P):
                nc.sync.dma_start(out=buf_d[i * P:(i + 1) * P, :], in_=zi[:])
            zb = wp.tile([P, C], BF16, tag="zb")
            nc.vector.memset(zb[:], 0.0)
            nc.sync.dma_start(out=agout[E * CAP:E * CAP + P, :], in_=zb[:])

            # ================= Phase A: local router logits =================
            with (
                tc.tile_pool(name="rpool", bufs=2) as rp,
                tc.tile_pool(name="rps", bufs=2, space="PSUM") as rps,
            ):
                wrs = rp.tile([P, 64], F32, tag="wr")
                nc.sync.dma_start(out=wrs[:], in_=wr[:])
                brs = rp.tile([1, E], F32, tag="brr")
                nc.sync.dma_start(out=brs[:], in_=brr[:])
                xts = []
                for k in range(C // P):
                    t = rp.tile([P, 1024], F32, tag=f"xt_{k}")
                    nc.sync.dma_start(out=t[:], in_=xt[k * P:(k + 1) * P, :])
                    xts.append(t)
                for m in range(1024 // P):
                    ps = rps.tile([P, E], F32, tag="lg")
                    for k in range(C // P):
                        nc.tensor.matmul(
                            ps[:], lhsT=xts[k][:, m * P:(m + 1) * P],
                            rhs=wrs[:, k * E:(k + 1) * E],
                            start=(k == 0), stop=False)
                    nc.tensor.matmul(ps[:], lhsT=on1[:], rhs=brs[:],
                                     start=False, stop=True)
                    lgs = rp.tile([P, E], F32, tag="lgout")
                    nc.scalar.activation(lgs[:], ps[:], AF.Copy)
                    nc.sync.dma_start(out=lgin[m * P:(m + 1) * P, :], in_=lgs[:])

            nc.gpsimd.collective_compute(
                "AllGather", OP.bypass, ins=[lgin[:]], outs=[lgout[:]],
                replica_groups=rg)
            if debug:
                nc.sync.dma_start(out=dbg_lg[:], in_=lgout[:])

            # ================= Phase B: full routing ========================
            with (
                tc.tile_pool(name="bpool", bufs=1) as bp,
                tc.tile_pool(name="bps", bufs=2, space="PSUM") as bps,
            ):
                W = NT * E  # 512
                L = bp.tile([P, W], F32, tag="L")
                nc.sync.dma_start(
                    out=_r3(L[:]),
                    in_=lgout[:].rearrange("(p n) e -> p n e", p=P))
                bv = bp.tile([P, W], F32, tag="binv")
                nc.sync.dma_start(out=bv[:], in_=binv[:])
                cm = bp.tile([P, W], F32, tag="cmask")
                nc.sync.dma_start(out=cm[:], in_=cmask[:])
                trs = bp.tile([P, P], F32, tag="tri")
                nc.sync.dma_start(out=trs[:], in_=tri[:])

                def tt(o, a, b, op):
                    nc.vector.tensor_tensor(out=o, in0=a, in1=b, op=op)

                v1 = bp.tile([P, NT], F32, tag="v1")
                nc.vector.reduce_max(v1[:], _r3(L[:]), axis=AX.X)
                m1 = bp.tile([P, W], F32, tag="t0")
                tt(_r3(m1[:]), _r3(L[:]), v1[:].to_broadcast([P, NT, E]),
                   OP.is_equal)
                tmp = bp.tile([P, W], F32, tag="t1")
                tt(tmp[:], m1[:], bv[:], OP.mult)
                e1x = bp.tile([P, NT], F32, tag="e1x")
                nc.vector.reduce_max(e1x[:], _r3(tmp[:]), axis=AX.X)
                oh1 = bp.tile([P, W], F32, tag="oh1")
                tt(_r3(oh1[:]), _r3(bv[:]), e1x[:].to_broadcast([P, NT, E]),
                   OP.is_equal)
                msk = bp.tile([P, W], F32, tag="t0b")
                nc.vector.tensor_scalar(out=msk[:], in0=oh1[:], scalar1=-BIG,
                                        scalar2=None, op0=OP.mult)
                tt(msk[:], L[:], msk[:], OP.add)
                v2 = bp.tile([P, NT], F32, tag="v2")
                nc.vector.reduce_max(v2[:], _r3(msk[:]), axis=AX.X)
                m2 = bp.tile([P, W], F32, tag="t2")
                tt(_r3(m2[:]), _r3(msk[:]), v2[:].to_broadcast([P, NT, E]),
                   OP.is_equal)
                tt(m2[:], m2[:], bv[:], OP.mult)
                e2x = bp.tile([P, NT], F32, tag="e2x")
                nc.vector.reduce_max(e2x[:], _r3(m2[:]), axis=AX.X)
                oh2 = bp.tile([P, W], F32, tag="oh2")
                tt(_r3(oh2[:]), _r3(bv[:]), e2x[:].to_broadcast([P, NT, E]),
                   OP.is_equal)
                sel = bp.tile([P, W], F32, tag="sel")
                tt(sel[:], oh1[:], oh2[:], OP.add)

                vd = bp.tile([P, NT], F32, tag="vd")
                tt(vd[:], v1[:], v2[:], OP.subtract)
                g1 = bp.tile([P, NT], F32, tag="g1")
                nc.scalar.activation(g1[:], vd[:], AF.Sigmoid)
                g2 = bp.tile([P, NT], F32, tag="g2")
                nc.vector.tensor_scalar(out=g2[:], in0=g1[:], scalar1=-1.0,
                                        scalar2=1.0, op0=OP.mult, op1=OP.add)

                # inclusive prefix over n (shift s tokens == 8s columns)
                cur = sel
                pidx = 0
                for s in (1, 2, 4, 8, 16, 32):
                    nxt = bp.tile([P, W], F32, tag=f"pf{pidx % 2}")
                    pidx += 1
                    tt(nxt[:, 8 * s:], cur[:, 8 * s:], cur[:, :W - 8 * s], OP.add)
                    nc.vector.tensor_copy(out=nxt[:, :8 * s], in_=cur[:, :8 * s])
                    cur = nxt
                incl = cur
                offp = bps.tile([P, E], F32, tag="offp")
                nc.tensor.matmul(offp[:], lhsT=trs[:], rhs=incl[:, W - E:W],
                                 start=True, stop=True)
                offs = bp.tile([P, E], F32, tag="offs")
                nc.scalar.activation(offs[:], offp[:], AF.Copy)
                rank = bp.tile([P, W], F32, tag="rank")
                tt(rank[:], incl[:], sel[:], OP.subtract)
                offs3 = bass.AP(
                    offs[:].tensor, offs[:].offset,
                    [offs[:].ap[0], [0, NT], offs[:].ap[1]])
                tt(_r3(rank[:]), _r3(rank[:]), offs3, OP.add)

                # dispatch indices for my expert: disp = rank + (1-sel)*BIG,
                # clamped to the dump row CAP, as int16 token-major
                disp = bp.tile([P, W], F32, tag="disp")
                nc.vector.tensor_scalar(out=disp[:], in0=sel[:], scalar1=-BIG,
                                        scalar2=BIG, op0=OP.mult, op1=OP.add)
                tt(disp[:], rank[:], disp[:], OP.add)
                dcf = bp.tile([P, W], F32, tag="dcf")
                tt(dcf[:], disp[:], cm[:], OP.mult)
                dce = bp.tile([P, NT], F32, tag="dce")
                nc.vector.reduce_sum(dce[:], _r3(dcf[:]), axis=AX.X)
                dcc = bp.tile([P, NT], F32, tag="dcc")
                nc.vector.tensor_scalar(out=dcc[:], in0=dce[:],
                                        scalar1=float(CAP), scalar2=None,
                                        op0=OP.min)
                dci = bp.tile([P, NT], mybir.dt.int16, tag="dci")
                nc.vector.tensor_copy(out=dci[:], in_=dcc[:])
                # bounce token-major to DRAM, reload in the 16-partition
                # index-wrap layout the batched SWDGE ops expect, replicate to
                # all 8 Q7 core blocks
                nc.sync.dma_start(
                    out=didx_d[:].rearrange("(p n) -> p n", p=P), in_=dci[:])
                didx = wp.tile([P, W], mybir.dt.int16, tag="didx")
                nc.sync.dma_start(
                    out=didx[0:16, :].rearrange("q (n b) -> q n b", b=8),
                    in_=didx_d[:].rearrange("(b q n) -> q n b", q=16, n=NT))
                for r in range(1, 8):
                    nc.sync.dma_start(out=didx[16 * r:16 * (r + 1), :],
                                      in_=didx[0:16, :])

                # combine indices, chunk-strided AG layout:
                # idx = (r>>9)*4096 + e*512 + (r&511)  (+BIG if dropped/unsel)
                def mkidx(ohx, exx, tag):
                    ei = bp.tile([P, NT], F32, tag=f"ei{tag}")
                    nc.vector.tensor_scalar(out=ei[:], in0=exx[:], scalar1=-1.0,
                                            scalar2=BIG, op0=OP.mult, op1=OP.add)
                    tmpr = bp.tile([P, W], F32, tag="t3")
                    tt(tmpr[:], rank[:], ohx[:], OP.mult)
                    ri = bp.tile([P, NT], F32, tag=f"ri{tag}")
                    nc.vector.reduce_sum(ri[:], _r3(tmpr[:]), axis=AX.X)
                    rii = bp.tile([P, NT], I32, tag=f"rii{tag}")
                    nc.vector.tensor_copy(out=rii[:], in_=ri[:])
                    rc = bp.tile([P, NT], I32, tag=f"rc{tag}")
                    nc.vector.tensor_scalar(out=rc[:], in0=rii[:], scalar1=9,
                                            scalar2=12, op0=OP.arith_shift_right,
                                            op1=OP.logical_shift_left)
                    rl = bp.tile([P, NT], I32, tag=f"rl{tag}")
                    nc.vector.tensor_scalar(out=rl[:], in0=rii[:], scalar1=511,
                                            scalar2=None, op0=OP.bitwise_and)
                    tt(rc[:], rc[:], rl[:], OP.add)
                    # dropped: r >= CAP -> +BIG
                    di = bp.tile([P, NT], F32, tag=f"di{tag}")
                    nc.vector.tensor_scalar(out=di[:], in0=ri[:],
                                            scalar1=float(CAP) - 0.5,
                                            scalar2=BIG,
                                            op0=OP.is_gt, op1=OP.mult)
                    ix = bp.tile([P, NT], F32, tag=f"ix{tag}")
                    nc.vector.tensor_scalar(out=ix[:], in0=ei[:],
                                            scalar1=512.0, scalar2=None,
                                            op0=OP.mult)
                    tt(ix[:], ix[:], di[:], OP.add)
                    rcf = bp.tile([P, NT], F32, tag=f"rcf{tag}")
                    nc.vector.tensor_copy(out=rcf[:], in_=rc[:])
                    tt(ix[:], ix[:], rcf[:], OP.add)
                    # dropped contributions point at the zeroed pad row
                    nc.vector.tensor_scalar(out=ix[:], in0=ix[:],
                                            scalar1=float(E * CAP),
                                            scalar2=None, op0=OP.min)
                    return ix

                ix1 = mkidx(oh1, e1x, "1")
                ix2 = mkidx(oh2, e2x, "2")

                pk = bp.tile([P, NT * 4], F32, tag="pk")
                pk4 = pk[:].rearrange("p (n f) -> p n f", f=4)
                nc.vector.tensor_copy(out=pk4[:, :, 0], in_=ix1[:])
                nc.vector.tensor_copy(out=pk4[:, :, 1], in_=ix2[:])
                nc.vector.tensor_copy(out=pk4[:, :, 2], in_=g1[:])
                nc.vector.tensor_copy(out=pk4[:, :, 3], in_=g2[:])
                nc.sync.dma_start(
                    out=rsc[:].rearrange("(p q) f -> p q f", p=P), in_=pk4)
                if debug:
                    nc.sync.dma_start(out=dbg_rs[:], in_=rsc[:])

                # ============== Phase C: dispatch scatter ===================
                # own-token combine metadata prefetch (independent of AGs)
                own = wp.tile([P, 32], F32, tag="own")
                nc.gpsimd.indirect_dma_start(
                    out=own[:], out_offset=None,
                    in_=rsc[:],
                    in_offset=bass.IndirectOffsetOnAxis(ap=cis[:, :1], axis=0),
                )
                own4 = own[:].rearrange("p (n f) -> p n f", f=4)
                i1 = wp.tile([P, 8], mybir.dt.int16, tag="i1")
                nc.vector.tensor_copy(out=i1[:], in_=own4[:, :, 0])
                i2 = wp.tile([P, 8], mybir.dt.int16, tag="i2")
                nc.vector.tensor_copy(out=i2[:], in_=own4[:, :, 1])
                go1 = wp.tile([P, 8], F32, tag="go1")
                nc.vector.tensor_copy(out=go1[:], in_=own4[:, :, 2])
                go2 = wp.tile([P, 8], F32, tag="go2")
                nc.vector.tensor_copy(out=go2[:], in_=own4[:, :, 3])
                # combine-gather index tiles: entry i = p' + 128*n' for own
                # token j = p'*8 + n', in 16-partition wrap, replicated
                cidx = []
                for f, it in ((0, i1), (1, i2)):
                    nc.sync.dma_start(
                        out=cscr[f, :].rearrange("(p n) -> p n", p=P), in_=it[:])
                    ci = wp.tile([P, 64], mybir.dt.int16, tag=f"cidx{f}")
                    nc.sync.dma_start(
                        out=ci[0:16, :].rearrange("q (n b) -> q n b", b=8),
                        in_=cscr[f, :].rearrange("(b q n) -> q n b", q=16, n=8))
                    for r in range(1, 8):
                        nc.sync.dma_start(out=ci[16 * r:16 * (r + 1), :],
                                          in_=ci[0:16, :])
                    cidx.append(ci)

                # scatter token ids into buf_d[slot] (dump row absorbs the
                # rest); scatter-add requires 256B elements, so each id is
                # replicated across a 128-wide int16 row
                tkb = bp.tile([P, NT * P], mybir.dt.int16, tag="tkb")
                nc.vector.tensor_copy(
                    out=tkb[:].rearrange("p (n e) -> p n e", e=P),
                    in_=tks[:].to_broadcast([P, NT, P]))
                tkb3 = tkb[:].rearrange("p (n e) -> p n e", e=P)
                for k in range(NTOK // 512):
                    nc.gpsimd.dma_scatter_add(
                        out_ap=buf_d[:],
                        in_ap=tkb3[:, k * 4:(k + 1) * 4, :],
                        idxs_ap=didx[:, k * 32:(k + 1) * 32],
                        num_idxs=512,
                        num_idxs_reg=512,
                        elem_size=P,
                        single_packet=False,
                    )
                # reload buf (slot -> token id) in index-wrap layout
                bufi = wp.tile([P, CAP // 16], mybir.dt.int16, tag="bufi")
                nc.sync.dma_start(
                    out=bufi[0:16, :],
                    in_=buf_d[0:CAP, 0:1].rearrange(
                        "(s q) one -> q (s one)", q=16))
                for r in range(1, 8):
                    nc.sync.dma_start(out=bufi[16 * r:16 * (r + 1), :],
                                      in_=bufi[0:16, :])

            # ---- weights (loaded behind router/dispatch in priority order) --
            w1t = []
            for k in range(C // P):
                t = wp.tile([P, HD], BF16, tag=f"w1_{k}")
                nc.sync.dma_start(out=t[:], in_=w1[k * P:(k + 1) * P, :])
                w1t.append(t)
            w2t = []
            for k in range(HD // P):
                t = wp.tile([P, C], BF16, tag=f"w2_{k}")
                nc.sync.dma_start(out=t[:], in_=w2[k * P:(k + 1) * P, :])
                w2t.append(t)
            b1s = wp.tile([P, HD // P], F32, tag="b1a")
            nc.sync.dma_start(out=b1s[:], in_=b1a[:])
            b2s = wp.tile([1, C], F32, tag="b2r")
            nc.sync.dma_start(out=b2s[:], in_=b2r[:])

            # ================= Phase D: expert FFN ==========================
            with (
                tc.tile_pool(name="fpool", bufs=1) as fp,
                tc.tile_pool(name="fps", bufs=4, space="PSUM") as fpsH,
                tc.tile_pool(name="fps2", bufs=2, space="PSUM") as fpsM,
                tc.tile_pool(name="xgt", bufs=2) as xp,
                tc.tile_pool(name="osb", bufs=3) as op_,
            ):
                for chk in range(NCH):
                    gout = xp.tile([P, (TCH // P) * C], BF16, tag="gout")
                    gout3 = gout[:].rearrange("p (a c) -> p a c", c=C)
                    nc.gpsimd.dma_gather(
                        out_ap=gout3,
                        in_ap=xb[:],
                        idxs_ap=bufi[:, chk * (TCH // 16):(chk + 1) * (TCH // 16)],
                        num_idxs=TCH,
                        num_idxs_reg=TCH,
                        elem_size=C,
                        transpose=False,
                        single_packet=False,
                    )
                    nc.sync.dma_start(
                        out=xg[chk * TCH:(chk + 1) * TCH, :].rearrange(
                            "(a p) c -> p a c", p=P),
                        in_=gout3)
                    xgt = xp.tile([P, (C // P) * TCH], BF16, tag="xgt")
                    xgt3 = xgt[:].rearrange("p (k s) -> p k s", s=TCH)
                    for k in range(C // P):
                        nc.scalar.dma_start(
                            out=xgt3[:, k, :],
                            in_=xg[chk * TCH:(chk + 1) * TCH, k * P:(k + 1) * P],
                            transpose=True)
                    hs = []
                    for m in range(HD // P):
                        ps = fpsH.tile([P, TCH], F32, tag="hps")
                        for k in range(C // P):
                            nc.tensor.matmul(
                                ps[:], lhsT=w1t[k][:, m * P:(m + 1) * P],
                                rhs=xgt3[:, k, :], start=(k == 0),
                                stop=(k == C // P - 1))
                        h = fp.tile([P, TCH], BF16, tag=f"h_{chk % 2}_{m}")
                        nc.scalar.activation(h[:], ps[:], AF.Gelu,
                                             bias=b1s[:, m:m + 1])
                        hs.append(h)
                    for st in range(TCH // P):
                        for cf in range(2):
                            ps2 = fpsM.tile([P, 512], F32, tag="ops")
                            for k in range(HD // P):
                                nc.tensor.matmul(
                                    ps2[:], lhsT=hs[k][:, st * P:(st + 1) * P],
                                    rhs=w2t[k][:, cf * 512:(cf + 1) * 512],
                                    start=(k == 0), stop=False)
                            nc.tensor.matmul(
                                ps2[:], lhsT=on1[:],
                                rhs=b2s[:, cf * 512:(cf + 1) * 512],
                                start=False, stop=True)
                            ob = op_.tile([P, 512], BF16, tag="ob")
                            nc.scalar.activation(ob[:], ps2[:], AF.Copy)
                            nc.sync.dma_start(
                                out=agin[chk * TCH + st * P:
                                         chk * TCH + (st + 1) * P,
                                         cf * 512:(cf + 1) * 512],
                                in_=ob[:])
                    # ---- chunked output AllGather (overlaps later chunks) ---
                    if (chk + 1) % (AGCH // TCH) == 0:
                        ag = (chk + 1) // (AGCH // TCH) - 1
                        nc.gpsimd.collective_compute(
                            "AllGather", OP.bypass,
                            ins=[agin[ag * AGCH:(ag + 1) * AGCH, :]],
                            outs=[agout[ag * E * AGCH:(ag + 1) * E * AGCH, :]],
                            replica_groups=rg)

            if debug:
                dbx = wp.tile([P, C], F32, tag="dbx")
                dbb = wp.tile([P, P], mybir.dt.int16, tag="dbb")
                dbf = wp.tile([P, P], F32, tag="dbf")
                for i in range((CAP + P) // P):
                    nc.sync.dma_start(out=dbb[:], in_=buf_d[i * P:(i + 1) * P, :])
                    nc.vector.tensor_copy(out=dbf[:], in_=dbb[:])
                    nc.sync.dma_start(out=dbg_bf[i * P:(i + 1) * P, :], in_=dbf[:])
                for i in range(E * CAP // P):
                    nc.sync.dma_start(out=dbx[:], in_=agout[i * P:(i + 1) * P, :])
                    nc.sync.dma_start(out=dbg_ag[i * P:(i + 1) * P, :], in_=dbx[:])

            # ================= Phase F: combine own tokens ==================
            with tc.tile_pool(name="cpool", bufs=1) as cp:
                cg = []
                for f in range(2):
                    halves = []
                    for h in range(2):
                        t = cp.tile([P, 4 * C], BF16, tag=f"cg{f}_{h}")
                        nc.gpsimd.dma_gather(
                            out_ap=t[:].rearrange("p (n c) -> p n c", c=C),
                            in_ap=agout[:],
                            idxs_ap=cidx[f][:, h * 32:(h + 1) * 32],
                            num_idxs=512,
                            num_idxs_reg=512,
                            elem_size=C,
                            transpose=False,
                            single_packet=False,
                        )
                        halves.append(t[:].rearrange("p (n c) -> p n c", c=C))
                    cg.append(halves)
                with tc.tile_pool(name="copool", bufs=3) as cop:
                    for j in range(8):
                        h, jj = j // 4, j % 4
                        o1 = cop.tile([P, C], F32, tag="o1")
                        nc.vector.tensor_scalar(out=o1[:], in0=cg[0][h][:, jj, :],
                                                scalar1=go1[:, j:j + 1],
                                                scalar2=None, op0=OP.mult)
                        o2 = cop.tile([P, C], F32, tag="o2")
                        nc.vector.tensor_scalar(out=o2[:], in0=cg[1][h][:, jj, :],
                                                scalar1=go2[:, j:j + 1],
                                                scalar2=None, op0=OP.mult)
                        nc.vector.tensor_tensor(out=o1[:], in0=o1[:], in1=o2[:],
                                                op=OP.add)
                        nc.sync.dma_start(
                            out=out[:].rearrange("(p n) c -> p n c", p=P)[:, j, :],
                            in_=o1[:])

    nc.compile()
    return nc


def _host_inputs(x, Wr, br, W1, b1, W2, b2):
    xf = np.ascontiguousarray(x.reshape(NTOK, C)).astype(np.float32)
    xb = xf.astype(ml_dtypes.bfloat16)
    wr_a = np.ascontiguousarray(
        Wr.reshape(C // P, P, E).transpose(1, 0, 2).reshape(P, 64)).astype(
            np.float32)
    brr = br.reshape(1, E).astype(np.float32)
    tri = np.triu(np.ones((P, P), np.float32), 1)
    binv = np.broadcast_to(
        np.tile(BIG - np.arange(E, dtype=np.float32), NT), (P, NT * E)).copy()
    ones1 = np.ones((1, P), np.float32)
    tokid = (np.arange(P, dtype=np.int32)[:, None] * NT
             + np.arange(NT, dtype=np.int32)[None, :]).astype(np.int16)
    in_maps = []
    for c in range(NCORES):
        cm = np.zeros(E, np.float32)
        cm[c] = 1.0
        cmask = np.broadcast_to(np.tile(cm, NT), (P, NT * E)).copy()
        in_maps.append({
            "xt": np.ascontiguousarray(xf[c * 1024:(c + 1) * 1024, :].T),
            "xb": xb,
            "wr": wr_a,
            "brr": brr,
            "w1": np.ascontiguousarray(W1[c]).astype(ml_dtypes.bfloat16),
            "b1a": np.ascontiguousarray(
                b1[c].reshape(HD // P, P).T).astype(np.float32),
            "w2": np.ascontiguousarray(W2[c]).astype(ml_dtypes.bfloat16),
            "b2r": b2[c].reshape(1, C).astype(np.float32),
            "tri": tri,
            "binv": binv,
            "cmask": cmask,
            "ones1": ones1,
            "cidp": (c * P + np.arange(P, dtype=np.int32)).reshape(P, 1),
            "tokid": tokid,
        })
    return in_maps


def _ensure_ntff_hook():
    """The agent image's antenv lacks axon_hooks; shim it so trace=True works."""
    import types
    try:
        import antenv.axon_hooks  # noqa: F401
        return
    except ImportError:
        pass
    import antenv
    mod = types.ModuleType("antenv.axon_hooks")
    state = {"h": None}
    mod.set_axon_ntff_profile_hook = lambda h: state.__setitem__("h", h)
    mod.get_axon_ntff_profile_hook = lambda: state["h"]
    sys.modules["antenv.axon_hooks"] = mod
    antenv.axon_hooks = mod
    from trn_agent_boot.trn_boot import _ntff_profile_via_ctypes
    mod.set_axon_ntff_profile_hook(
        _ntff_profile_via_ctypes("/opt/axon/libaxon_pjrt.so"))


def kernel(x, Wr, br, W1, b1, W2, b2, _debug=False, _trace=False):
    global _BUILT
    x, Wr, br = np.asarray(x), np.asarray(Wr), np.asarray(br)
    W1, b1, W2, b2 = map(np.asarray, (W1, b1, W2, b2))
    if _BUILT is None or _BUILT[1] != _debug:
        _BUILT = (build(debug=_debug), _debug)
    nc = _BUILT[0]
    in_maps = _host_inputs(x, Wr, br, W1, b1, W2, b2)
    if _trace:
        _ensure_ntff_hook()
    res = run_bass_kernel_spmd(nc, in_maps, list(range(NCORES)), trace=_trace)
    outs = np.concatenate([res.results[c]["out"] for c in range(NCORES)], 0)
    out = outs.reshape(x.shape).astype(np.float32)
    if _debug:
        kernel.debug_results = res
    if _trace:
        kernel.trace_results = res
    return out


# revision 12
# speedup vs baseline: 1.3470x; 1.3470x over previous
"""Trainium2 Bass kernel for BoringFeedForwardMOE (E=8 experts, top-2, cap=2048).

Strategy: expert parallelism across 8 NeuronCores.
  - Router computed data-parallel (each core: logits for its 1024-token slice,
    f32 to match reference top-k decisions bit-for-bit), AllGather of logits.
  - Full routing (top-2 + softmax gates + capacity ranks) recomputed identically
    on every core from the gathered [8192, 8] logits; prefix sums over tokens
    via log-step shifted adds (free axis) + strictly-upper-triangular matmul
    (partition axis).
  - Dispatch: one batched dma_scatter_add builds buf (slot -> token id) in
    DRAM (unselected/over-capacity entries land in a dump row); the FFN input
    is then produced chunk-by-chunk with dma_gather(transpose=True), which
    reads each slot's bf16 x-row from HBM directly into [feature, slot]
    layout for the TensorEngine.
  - FFN in bf16, gelu + biases fused. Outputs are AllGathered in 4 chunks of
    512 slots so the collective overlaps the FFN of later chunks.
  - Combine: two batched dma_gathers fetch each own token's <=2 expert
    contributions (dropped ones hit a zeroed pad row), f32 gates, write out.

Token layout on device: t = p*64 + n (p: partition 0..127, n: 0..63).
Core c owns tokens [c*1024, (c+1)*1024) and expert e = c.
agout row for contribution (e, r): (r>>9)*4096 + e*512 + (r & 511).
"""
import sys

if "/opt/trn_rl_repo" not in sys.path:
    sys.path.insert(0, "/opt/trn_rl_repo")

import numpy as np
import ml_dtypes

import concourse.bass as bass
import concourse.bacc as bacc
import concourse.mybir as mybir
from concourse.tile import TileContext
from concourse.bass_utils import run_bass_kernel_spmd

F32 = mybir.dt.float32
BF16 = mybir.dt.bfloat16
I32 = mybir.dt.int32
AF = mybir.ActivationFunctionType
OP = mybir.AluOpType
AX = mybir.AxisListType

NCORES = 8
P = 128
NTOK = 8192          # B*T
C = 1024
HD = 4096
E = 8
CAP = 2048
NT = NTOK // P       # 64 token columns per partition
BIG = 1.0e6
TCH = 256            # FFN slot chunk (per PSUM group)
NCH = CAP // TCH     # 8 chunks
AGCH = 512           # slots per output-AllGather chunk
NAG = CAP // AGCH    # 4 collectives

_BUILT = None


def _r3(ap, e=E):
    return ap.rearrange("p (n e) -> p n e", e=e)


def build(debug=False):
    nc = bacc.Bacc()

    # ---- per-core parameters -------------------------------------------------
    xt = nc.declare_dram_parameter("xt", [C, 1024], F32, isOutput=False)
    xb = nc.declare_dram_parameter("xb", [NTOK, C], BF16, isOutput=False)
    wr = nc.declare_dram_parameter("wr", [P, 64], F32, isOutput=False)
    brr = nc.declare_dram_parameter("brr", [1, E], F32, isOutput=False)
    w1 = nc.declare_dram_parameter("w1", [C, HD], BF16, isOutput=False)
    b1a = nc.declare_dram_parameter("b1a", [P, HD // P], F32, isOutput=False)
    w2 = nc.declare_dram_parameter("w2", [HD, C], BF16, isOutput=False)
    b2r = nc.declare_dram_parameter("b2r", [1, C], F32, isOutput=False)
    tri = nc.declare_dram_parameter("tri", [P, P], F32, isOutput=False)
    binv = nc.declare_dram_parameter("binv", [P, NT * E], F32, isOutput=False)
    cmask = nc.declare_dram_parameter("cmask", [P, NT * E], F32, isOutput=False)
    ones1 = nc.declare_dram_parameter("ones1", [1, P], F32, isOutput=False)
    cidp = nc.declare_dram_parameter("cidp", [P, 1], I32, isOutput=False)
    tokid = nc.declare_dram_parameter("tokid", [P, NT], mybir.dt.int16,
                                      isOutput=False)
    dmpr = nc.declare_dram_parameter("dmpr", [P, NT], F32, isOutput=False)
    out = nc.declare_dram_parameter("out", [1024, C], F32, isOutput=True)
    if debug:
        dbg_lg = nc.declare_dram_parameter("dbg_lg", [NTOK, E], F32, isOutput=True)
        dbg_rs = nc.declare_dram_parameter("dbg_rs", [1024, 32], F32, isOutput=True)
        dbg_bf = nc.declare_dram_parameter("dbg_bf", [CAP + P, P], F32, isOutput=True)
        dbg_ag = nc.declare_dram_parameter("dbg_ag", [E * CAP, C], F32, isOutput=True)

    # ---- internal DRAM -------------------------------------------------------
    lgin = nc.dram_tensor("lgin", [1024, E], F32)
    lgout = nc.dram_tensor("lgout", [NTOK, E], F32, addr_space="Shared")
    # buf_d row s (first int16) = token id at slot s of this expert; row CAP is
    # the dump row for unselected / over-capacity entries.
    buf_d = nc.dram_tensor("buf_d", [CAP + P, P], mybir.dt.int16)
    xg = nc.dram_tensor("xg", [CAP, C], BF16)
    didx_d = nc.dram_tensor("didx_d", [NTOK], mybir.dt.int16)
    cscr = nc.dram_tensor("cscr", [2, 1024], mybir.dt.int16)
    agin = nc.dram_tensor("agin", [CAP, C], BF16)
    # chunk k of agin ([AGCH, C] slots) -> agout rows [k*E*AGCH, (k+1)*E*AGCH);
    # rows [E*CAP, E*CAP+P) are a zeroed pad target for dropped contributions.
    agout = nc.dram_tensor("agout", [E * CAP + P, C], BF16, addr_space="Shared")
    rsc = nc.dram_tensor("rsc", [NTOK // 8, 32], F32)   # packed idx1/idx2/g1/g2

    rg = [list(range(NCORES))]

    with TileContext(nc) as tc:
        with tc.tile_pool(name="wpool", bufs=1) as wp:
            on1 = wp.tile([1, P], F32, tag="ones1")
            nc.sync.dma_start(out=on1[:], in_=ones1[:])
            cis = wp.tile([P, 1], I32, tag="cidp")
            nc.sync.dma_start(out=cis[:], in_=cidp[:])
            tks = wp.tile([P, NT], mybir.dt.int16, tag="tokid")
            nc.sync.dma_start(out=tks[:], in_=tokid[:])
            dms = wp.tile([P, NT], F32, tag="dmpr")
            nc.sync.dma_start(out=dms[:], in_=dmpr[:])
            # zero-fill buf_d (scatter-add target) and the agout pad rows
            zi = wp.tile([P, P], mybir.dt.int16, tag="zi")
            nc.vector.memset(zi[:], 0)
            for i in range((CAP + P) // P):
                nc.sync.dma_start(out=buf_d[i * P:(i + 1) * P, :], in_=zi[:])
            zb = wp.tile([P, C], BF16, tag="zb")
            nc.vector.memset(zb[:], 0.0)
            nc.sync.dma_start(out=agout[E * CAP:E * CAP + P, :], in_=zb[:])

            # ================= Phase A: local router logits =================
            with (
                tc.tile_pool(name="rpool", bufs=2) as rp,
                tc.tile_pool(name="rps", bufs=2, space="PSUM") as rps,
            ):
                wrs = rp.tile([P, 64], F32, tag="wr")
                nc.sync.dma_start(out=wrs[:], in_=wr[:])
                brs = rp.tile([1, E], F32, tag="brr")
                nc.sync.dma_start(out=brs[:], in_=brr[:])
                xts = []
                for k in range(C // P):
                    t = rp.tile([P, 1024], F32, tag=f"xt_{k}")
                    nc.sync.dma_start(out=t[:], in_=xt[k * P:(k + 1) * P, :])
                    xts.append(t)
                for m in range(1024 // P):
                    ps = rps.tile([P, E], F32, tag="lg")
                    for k in range(C // P):
                        nc.tensor.matmul(
                            ps[:], lhsT=xts[k][:, m * P:(m + 1) * P],
                            rhs=wrs[:, k * E:(k + 1) * E],
                            start=(k == 0), stop=False)
                    nc.tensor.matmul(ps[:], lhsT=on1[:], rhs=brs[:],
                                     start=False, stop=True)
                    lgs = rp.tile([P, E], F32, tag="lgout")
                    nc.scalar.activation(lgs[:], ps[:], AF.Copy)
                    nc.sync.dma_start(out=lgin[m * P:(m + 1) * P, :], in_=lgs[:])

            nc.gpsimd.collective_compute(
                "AllGather", OP.bypass, ins=[lgin[:]], outs=[lgout[:]],
                replica_groups=rg)
            if debug:
                nc.sync.dma_start(out=dbg_lg[:], in_=lgout[:])

            # ================= Phase B: full routing ========================
            with (
                tc.tile_pool(name="bpool", bufs=1) as bp,
                tc.tile_pool(name="bps", bufs=2, space="PSUM") as bps,
            ):
                W = NT * E  # 512
                L = bp.tile([P, W], F32, tag="L")
                nc.sync.dma_start(
                    out=_r3(L[:]),
                    in_=lgout[:].rearrange("(p n) e -> p n e", p=P))
                bv = bp.tile([P, W], F32, tag="binv")
                nc.sync.dma_start(out=bv[:], in_=binv[:])
                cm = bp.tile([P, W], F32, tag="cmask")
                nc.sync.dma_start(out=cm[:], in_=cmask[:])
                trs = bp.tile([P, P], F32, tag="tri")
                nc.sync.dma_start(out=trs[:], in_=tri[:])

                def tt(o, a, b, op):
                    nc.vector.tensor_tensor(out=o, in0=a, in1=b, op=op)

                v1 = bp.tile([P, NT], F32, tag="v1")
                nc.vector.reduce_max(v1[:], _r3(L[:]), axis=AX.X)
                m1 = bp.tile([P, W], F32, tag="t0")
                tt(_r3(m1[:]), _r3(L[:]), v1[:].to_broadcast([P, NT, E]),
                   OP.is_equal)
                tmp = bp.tile([P, W], F32, tag="t1")
                tt(tmp[:], m1[:], bv[:], OP.mult)
                e1x = bp.tile([P, NT], F32, tag="e1x")
                nc.vector.reduce_max(e1x[:], _r3(tmp[:]), axis=AX.X)
                oh1 = bp.tile([P, W], F32, tag="oh1")
                tt(_r3(oh1[:]), _r3(bv[:]), e1x[:].to_broadcast([P, NT, E]),
                   OP.is_equal)
                msk = bp.tile([P, W], F32, tag="t0b")
                nc.vector.tensor_scalar(out=msk[:], in0=oh1[:], scalar1=-BIG,
                                        scalar2=None, op0=OP.mult)
                tt(msk[:], L[:], msk[:], OP.add)
                v2 = bp.tile([P, NT], F32, tag="v2")
                nc.vector.reduce_max(v2[:], _r3(msk[:]), axis=AX.X)
                m2 = bp.tile([P, W], F32, tag="t2")
                tt(_r3(m2[:]), _r3(msk[:]), v2[:].to_broadcast([P, NT, E]),
                   OP.is_equal)
                tt(m2[:], m2[:], bv[:], OP.mult)
                e2x = bp.tile([P, NT], F32, tag="e2x")
                nc.vector.reduce_max(e2x[:], _r3(m2[:]), axis=AX.X)
                oh2 = bp.tile([P, W], F32, tag="oh2")
                tt(_r3(oh2[:]), _r3(bv[:]), e2x[:].to_broadcast([P, NT, E]),
                   OP.is_equal)
                sel = bp.tile([P, W], F32, tag="sel")
                tt(sel[:], oh1[:], oh2[:], OP.add)

                vd = bp.tile([P, NT], F32, tag="vd")
                tt(vd[:], v1[:], v2[:], OP.subtract)
                g1 = bp.tile([P, NT], F32, tag="g1")
                nc.scalar.activation(g1[:], vd[:], AF.Sigmoid)
                g2 = bp.tile([P, NT], F32, tag="g2")
                nc.vector.tensor_scalar(out=g2[:], in0=g1[:], scalar1=-1.0,
                                        scalar2=1.0, op0=OP.mult, op1=OP.add)

                # inclusive prefix over n (shift s tokens == 8s columns)
                cur = sel
                pidx = 0
                for s in (1, 2, 4, 8, 16, 32):
                    nxt = bp.tile([P, W], F32, tag=f"pf{pidx % 2}")
                    pidx += 1
                    tt(nxt[:, 8 * s:], cur[:, 8 * s:], cur[:, :W - 8 * s], OP.add)
                    nc.vector.tensor_copy(out=nxt[:, :8 * s], in_=cur[:, :8 * s])
                    cur = nxt
                incl = cur
                offp = bps.tile([P, E], F32, tag="offp")
                nc.tensor.matmul(offp[:], lhsT=trs[:], rhs=incl[:, W - E:W],
                                 start=True, stop=True)
                offs = bp.tile([P, E], F32, tag="offs")
                nc.scalar.activation(offs[:], offp[:], AF.Copy)
                rank = bp.tile([P, W], F32, tag="rank")
                tt(rank[:], incl[:], sel[:], OP.subtract)
                offs3 = bass.AP(
                    offs[:].tensor, offs[:].offset,
                    [offs[:].ap[0], [0, NT], offs[:].ap[1]])
                tt(_r3(rank[:]), _r3(rank[:]), offs3, OP.add)

                # dispatch indices for my expert: disp = rank + (1-sel)*BIG,
                # clamped to the dump row CAP, as int16 token-major
                disp = bp.tile([P, W], F32, tag="disp")
                nc.vector.tensor_scalar(out=disp[:], in0=sel[:], scalar1=-BIG,
                                        scalar2=BIG, op0=OP.mult, op1=OP.add)
                tt(disp[:], rank[:], disp[:], OP.add)
                dcf = bp.tile([P, W], F32, tag="dcf")
                tt(dcf[:], disp[:], cm[:], OP.mult)
                dce = bp.tile([P, NT], F32, tag="dce")
                nc.vector.reduce_sum(dce[:], _r3(dcf[:]), axis=AX.X)
                # clamp dropped entries onto per-token-spread dump rows so the
                # scatter-add's CCE RMW chains don't hammer one HBM row
                dcc = bp.tile([P, NT], F32, tag="dcc")
                tt(dcc[:], dce[:], dms[:], OP.min)
                dci = bp.tile([P, NT], mybir.dt.int16, tag="dci")
                nc.vector.tensor_copy(out=dci[:], in_=dcc[:])
                # bounce token-major to DRAM, reload in the 16-partition
                # index-wrap layout the batched SWDGE ops expect, replicate to
                # all 8 Q7 core blocks
                nc.sync.dma_start(
                    out=didx_d[:].rearrange("(p n) -> p n", p=P), in_=dci[:])
                didx = wp.tile([P, W], mybir.dt.int16, tag="didx")
                nc.sync.dma_start(
                    out=didx[0:16, :].rearrange("q (n b) -> q n b", b=8),
                    in_=didx_d[:].rearrange("(b q n) -> q n b", q=16, n=NT))
                for r in range(1, 8):
                    nc.sync.dma_start(out=didx[16 * r:16 * (r + 1), :],
                                      in_=didx[0:16, :])

                # combine indices, chunk-strided AG layout:
                # idx = (r>>9)*4096 + e*512 + (r&511)  (+BIG if dropped/unsel)
                def mkidx(ohx, exx, tag):
                    ei = bp.tile([P, NT], F32, tag=f"ei{tag}")
                    nc.vector.tensor_scalar(out=ei[:], in0=exx[:], scalar1=-1.0,
                                            scalar2=BIG, op0=OP.mult, op1=OP.add)
                    tmpr = bp.tile([P, W], F32, tag="t3")
                    tt(tmpr[:], rank[:], ohx[:], OP.mult)
                    ri = bp.tile([P, NT], F32, tag=f"ri{tag}")
                    nc.vector.reduce_sum(ri[:], _r3(tmpr[:]), axis=AX.X)
                    rii = bp.tile([P, NT], I32, tag=f"rii{tag}")
                    nc.vector.tensor_copy(out=rii[:], in_=ri[:])
                    rc = bp.tile([P, NT], I32, tag=f"rc{tag}")
                    nc.vector.tensor_scalar(out=rc[:], in0=rii[:], scalar1=9,
                                            scalar2=12, op0=OP.arith_shift_right,
                                            op1=OP.logical_shift_left)
                    rl = bp.tile([P, NT], I32, tag=f"rl{tag}")
                    nc.vector.tensor_scalar(out=rl[:], in0=rii[:], scalar1=511,
                                            scalar2=None, op0=OP.bitwise_and)
                    tt(rc[:], rc[:], rl[:], OP.add)
                    # dropped: r >= CAP -> +BIG
                    di = bp.tile([P, NT], F32, tag=f"di{tag}")
                    nc.vector.tensor_scalar(out=di[:], in0=ri[:],
                                            scalar1=float(CAP) - 0.5,
                                            scalar2=BIG,
                                            op0=OP.is_gt, op1=OP.mult)
                    ix = bp.tile([P, NT], F32, tag=f"ix{tag}")
                    nc.vector.tensor_scalar(out=ix[:], in0=ei[:],
                                            scalar1=512.0, scalar2=None,
                                            op0=OP.mult)
                    tt(ix[:], ix[:], di[:], OP.add)
                    rcf = bp.tile([P, NT], F32, tag=f"rcf{tag}")
                    nc.vector.tensor_copy(out=rcf[:], in_=rc[:])
                    tt(ix[:], ix[:], rcf[:], OP.add)
                    # dropped contributions point at the zeroed pad row
                    nc.vector.tensor_scalar(out=ix[:], in0=ix[:],
                                            scalar1=float(E * CAP),
                                            scalar2=None, op0=OP.min)
                    return ix

                ix1 = mkidx(oh1, e1x, "1")
                ix2 = mkidx(oh2, e2x, "2")

                pk = bp.tile([P, NT * 4], F32, tag="pk")
                pk4 = pk[:].rearrange("p (n f) -> p n f", f=4)
                nc.vector.tensor_copy(out=pk4[:, :, 0], in_=ix1[:])
                nc.vector.tensor_copy(out=pk4[:, :, 1], in_=ix2[:])
                nc.vector.tensor_copy(out=pk4[:, :, 2], in_=g1[:])
                nc.vector.tensor_copy(out=pk4[:, :, 3], in_=g2[:])
                nc.sync.dma_start(
                    out=rsc[:].rearrange("(p q) f -> p q f", p=P), in_=pk4)
                if debug:
                    nc.sync.dma_start(out=dbg_rs[:], in_=rsc[:])

                # ============== Phase C: dispatch scatter ===================
                # own-token combine metadata prefetch (independent of AGs)
                own = wp.tile([P, 32], F32, tag="own")
                nc.gpsimd.indirect_dma_start(
                    out=own[:], out_offset=None,
                    in_=rsc[:],
                    in_offset=bass.IndirectOffsetOnAxis(ap=cis[:, :1], axis=0),
                )
                own4 = own[:].rearrange("p (n f) -> p n f", f=4)
                i1 = wp.tile([P, 8], mybir.dt.int16, tag="i1")
                nc.vector.tensor_copy(out=i1[:], in_=own4[:, :, 0])
                i2 = wp.tile([P, 8], mybir.dt.int16, tag="i2")
                nc.vector.tensor_copy(out=i2[:], in_=own4[:, :, 1])
                go1 = wp.tile([P, 8], F32, tag="go1")
                nc.vector.tensor_copy(out=go1[:], in_=own4[:, :, 2])
                go2 = wp.tile([P, 8], F32, tag="go2")
                nc.vector.tensor_copy(out=go2[:], in_=own4[:, :, 3])
                # combine-gather index tiles: entry i = p' + 128*n' for own
                # token j = p'*8 + n', in 16-partition wrap, replicated
                cidx = []
                for f, it in ((0, i1), (1, i2)):
                    nc.sync.dma_start(
                        out=cscr[f, :].rearrange("(p n) -> p n", p=P), in_=it[:])
                    ci = wp.tile([P, 64], mybir.dt.int16, tag=f"cidx{f}")
                    nc.sync.dma_start(
                        out=ci[0:16, :].rearrange("q (n b) -> q n b", b=8),
                        in_=cscr[f, :].rearrange("(b q n) -> q n b", q=16, n=8))
                    for r in range(1, 8):
                        nc.sync.dma_start(out=ci[16 * r:16 * (r + 1), :],
                                          in_=ci[0:16, :])
                    cidx.append(ci)

                # scatter token ids into buf_d[slot] (dump row absorbs the
                # rest); scatter-add requires 256B elements, so each id is
                # replicated across a 128-wide int16 row
                tkb = bp.tile([P, NT * P], mybir.dt.int16, tag="tkb")
                nc.vector.tensor_copy(
                    out=tkb[:].rearrange("p (n e) -> p n e", e=P),
                    in_=tks[:].to_broadcast([P, NT, P]))
                tkb3 = tkb[:].rearrange("p (n e) -> p n e", e=P)
                for k in range(NTOK // 512):
                    nc.gpsimd.dma_scatter_add(
                        out_ap=buf_d[:],
                        in_ap=tkb3[:, k * 4:(k + 1) * 4, :],
                        idxs_ap=didx[:, k * 32:(k + 1) * 32],
                        num_idxs=512,
                        num_idxs_reg=512,
                        elem_size=P,
                        single_packet=False,
                    )
                # reload buf (slot -> token id) in index-wrap layout
                bufi = wp.tile([P, CAP // 16], mybir.dt.int16, tag="bufi")
                nc.sync.dma_start(
                    out=bufi[0:16, :],
                    in_=buf_d[0:CAP, 0:1].rearrange(
                        "(s q) one -> q (s one)", q=16))
                for r in range(1, 8):
                    nc.sync.dma_start(out=bufi[16 * r:16 * (r + 1), :],
                                      in_=bufi[0:16, :])

            # ---- weights (loaded behind router/dispatch in priority order) --
            w1t = []
            for k in range(C // P):
                t = wp.tile([P, HD], BF16, tag=f"w1_{k}")
                nc.sync.dma_start(out=t[:], in_=w1[k * P:(k + 1) * P, :])
                w1t.append(t)
            w2t = []
            for k in range(HD // P):
                t = wp.tile([P, C], BF16, tag=f"w2_{k}")
                nc.sync.dma_start(out=t[:], in_=w2[k * P:(k + 1) * P, :])
                w2t.append(t)
            b1s = wp.tile([P, HD // P], F32, tag="b1a")
            nc.sync.dma_start(out=b1s[:], in_=b1a[:])
            b2s = wp.tile([1, C], F32, tag="b2r")
            nc.sync.dma_start(out=b2s[:], in_=b2r[:])

            # ================= Phase D: expert FFN ==========================
            with (
                tc.tile_pool(name="fpool", bufs=1) as fp,
                tc.tile_pool(name="fps", bufs=4, space="PSUM") as fpsH,
                tc.tile_pool(name="fps2", bufs=2, space="PSUM") as fpsM,
                tc.tile_pool(name="xgt", bufs=2) as xp,
                tc.tile_pool(name="osb", bufs=3) as op_,
            ):
                for chk in range(NCH):
                    gout = xp.tile([P, (TCH // P) * C], BF16, tag="gout")
                    gout3 = gout[:].rearrange("p (a c) -> p a c", c=C)
                    nc.gpsimd.dma_gather(
                        out_ap=gout3,
                        in_ap=xb[:],
                        idxs_ap=bufi[:, chk * (TCH // 16):(chk + 1) * (TCH // 16)],
                        num_idxs=TCH,
                        num_idxs_reg=TCH,
                        elem_size=C,
                        transpose=False,
                        single_packet=False,
                    )
                    nc.sync.dma_start(
                        out=xg[chk * TCH:(chk + 1) * TCH, :].rearrange(
                            "(a p) c -> p a c", p=P),
                        in_=gout3)
                    xgt = xp.tile([P, (C // P) * TCH], BF16, tag="xgt")
                    xgt3 = xgt[:].rearrange("p (k s) -> p k s", s=TCH)
                    for k in range(C // P):
                        nc.scalar.dma_start(
                            out=xgt3[:, k, :],
                            in_=xg[chk * TCH:(chk + 1) * TCH, k * P:(k + 1) * P],
                            transpose=True)
                    hs = []
                    for m in range(HD // P):
                        ps = fpsH.tile([P, TCH], F32, tag="hps")
                        for k in range(C // P):
                            nc.tensor.matmul(
                                ps[:], lhsT=w1t[k][:, m * P:(m + 1) * P],
                                rhs=xgt3[:, k, :], start=(k == 0),
                                stop=(k == C // P - 1))
                        h = fp.tile([P, TCH], BF16, tag=f"h_{chk % 2}_{m}")
                        nc.scalar.activation(h[:], ps[:], AF.Gelu,
                                             bias=b1s[:, m:m + 1])
                        hs.append(h)
                    for st in range(TCH // P):
                        for cf in range(2):
                            ps2 = fpsM.tile([P, 512], F32, tag="ops")
                            for k in range(HD // P):
                                nc.tensor.matmul(
                                    ps2[:], lhsT=hs[k][:, st * P:(st + 1) * P],
                                    rhs=w2t[k][:, cf * 512:(cf + 1) * 512],
                                    start=(k == 0), stop=False)
                            nc.tensor.matmul(
                                ps2[:], lhsT=on1[:],
                                rhs=b2s[:, cf * 512:(cf + 1) * 512],
                                start=False, stop=True)
                            ob = op_.tile([P, 512], BF16, tag="ob")
                            nc.scalar.activation(ob[:], ps2[:], AF.Copy)
                            nc.sync.dma_start(
                                out=agin[chk * TCH + st * P:
                                         chk * TCH + (st + 1) * P,
                                         cf * 512:(cf + 1) * 512],
                                in_=ob[:])
                    # ---- chunked output AllGather (overlaps later chunks) ---
                    if (chk + 1) % (AGCH // TCH) == 0:
                        ag = (chk + 1) // (AGCH // TCH) - 1
                        nc.gpsimd.collective_compute(
                            "AllGather", OP.bypass,
                            ins=[agin[ag * AGCH:(ag + 1) * AGCH, :]],
                            outs=[agout[ag * E * AGCH:(ag + 1) * E * AGCH, :]],
                            replica_groups=rg)

            if debug:
                dbx = wp.tile([P, C], F32, tag="dbx")
                dbb = wp.tile([P, P], mybir.dt.int16, tag="dbb")
                dbf = wp.tile([P, P], F32, tag="dbf")
                for i in range((CAP + P) // P):
                    nc.sync.dma_start(out=dbb[:], in_=buf_d[i * P:(i + 1) * P, :])
                    nc.vector.tensor_copy(out=dbf[:], in_=dbb[:])
                    nc.sync.dma_start(out=dbg_bf[i * P:(i + 1) * P, :], in_=dbf[:])
                for i in range(E * CAP // P):
                    nc.sync.dma_start(out=dbx[:], in_=agout[i * P:(i + 1) * P, :])
                    nc.sync.dma_start(out=dbg_ag[i * P:(i + 1) * P, :], in_=dbx[:])

            # ================= Phase F: combine own tokens ==================
            with tc.tile_pool(name="cpool", bufs=1) as cp:
                cg = []
                for f in range(2):
                    halves = []
                    for h in range(2):
                        t = cp.tile([P, 4 * C], BF16, tag=f"cg{f}_{h}")
                        nc.gpsimd.dma_gather(
                            out_ap=t[:].rearrange("p (n c) -> p n c", c=C),
                            in_ap=agout[:],
                            idxs_ap=cidx[f][:, h * 32:(h + 1) * 32],
                            num_idxs=512,
                            num_idxs_reg=512,
                            elem_size=C,
                            transpose=False,
                            single_packet=False,
                        )
                        halves.append(t[:].rearrange("p (n c) -> p n c", c=C))
                    cg.append(halves)
                with tc.tile_pool(name="copool", bufs=3) as cop:
                    for j in range(8):
                        h, jj = j // 4, j % 4
                        o1 = cop.tile([P, C], F32, tag="o1")
                        nc.vector.tensor_scalar(out=o1[:], in0=cg[0][h][:, jj, :],
                                                scalar1=go1[:, j:j + 1],
                                                scalar2=None, op0=OP.mult)
                        o2 = cop.tile([P, C], F32, tag="o2")
                        nc.vector.tensor_scalar(out=o2[:], in0=cg[1][h][:, jj, :],
                                                scalar1=go2[:, j:j + 1],
                                                scalar2=None, op0=OP.mult)
                        nc.vector.tensor_tensor(out=o1[:], in0=o1[:], in1=o2[:],
                                                op=OP.add)
                        nc.sync.dma_start(
                            out=out[:].rearrange("(p n) c -> p n c", p=P)[:, j, :],
                            in_=o1[:])

    nc.compile()
    return nc


def _host_inputs(x, Wr, br, W1, b1, W2, b2):
    xf = np.ascontiguousarray(x.reshape(NTOK, C)).astype(np.float32)
    xb = xf.astype(ml_dtypes.bfloat16)
    wr_a = np.ascontiguousarray(
        Wr.reshape(C // P, P, E).transpose(1, 0, 2).reshape(P, 64)).astype(
            np.float32)
    brr = br.reshape(1, E).astype(np.float32)
    tri = np.triu(np.ones((P, P), np.float32), 1)
    binv = np.broadcast_to(
        np.tile(BIG - np.arange(E, dtype=np.float32), NT), (P, NT * E)).copy()
    ones1 = np.ones((1, P), np.float32)
    tokid = (np.arange(P, dtype=np.int32)[:, None] * NT
             + np.arange(NT, dtype=np.int32)[None, :]).astype(np.int16)
    dmpr = (CAP + (np.arange(P)[:, None] * NT + np.arange(NT)[None, :]) % P
            ).astype(np.float32)
    in_maps = []
    for c in range(NCORES):
        cm = np.zeros(E, np.float32)
        cm[c] = 1.0
        cmask = np.broadcast_to(np.tile(cm, NT), (P, NT * E)).copy()
        in_maps.append({
            "xt": np.ascontiguousarray(xf[c * 1024:(c + 1) * 1024, :].T),
            "xb": xb,
            "wr": wr_a,
            "brr": brr,
            "w1": np.ascontiguousarray(W1[c]).astype(ml_dtypes.bfloat16),
            "b1a": np.ascontiguousarray(
                b1[c].reshape(HD // P, P).T).astype(np.float32),
            "w2": np.ascontiguousarray(W2[c]).astype(ml_dtypes.bfloat16),
            "b2r": b2[c].reshape(1, C).astype(np.float32),
            "tri": tri,
            "binv": binv,
            "cmask": cmask,
            "ones1": ones1,
            "cidp": (c * P + np.arange(P, dtype=np.int32)).reshape(P, 1),
            "tokid": tokid,
            "dmpr": dmpr,
        })
    return in_maps


def _ensure_ntff_hook():
    """The agent image's antenv lacks axon_hooks; shim it so trace=True works."""
    import types
    try:
        import antenv.axon_hooks  # noqa: F401
        return
    except ImportError:
        pass
    import antenv
    mod = types.ModuleType("antenv.axon_hooks")
    state = {"h": None}
    mod.set_axon_ntff_profile_hook = lambda h: state.__setitem__("h", h)
    mod.get_axon_ntff_profile_hook = lambda: state["h"]
    sys.modules["antenv.axon_hooks"] = mod
    antenv.axon_hooks = mod
    from trn_agent_boot.trn_boot import _ntff_profile_via_ctypes
    mod.set_axon_ntff_profile_hook(
        _ntff_profile_via_ctypes("/opt/axon/libaxon_pjrt.so"))


def kernel(x, Wr, br, W1, b1, W2, b2, _debug=False, _trace=False):
    global _BUILT
    x, Wr, br = np.asarray(x), np.asarray(Wr), np.asarray(br)
    W1, b1, W2, b2 = map(np.asarray, (W1, b1, W2, b2))
    if _BUILT is None or _BUILT[1] != _debug:
        _BUILT = (build(debug=_debug), _debug)
    nc = _BUILT[0]
    in_maps = _host_inputs(x, Wr, br, W1, b1, W2, b2)
    if _trace:
        _ensure_ntff_hook()
    res = run_bass_kernel_spmd(nc, in_maps, list(range(NCORES)), trace=_trace)
    outs = np.concatenate([res.results[c]["out"] for c in range(NCORES)], 0)
    out = outs.reshape(x.shape).astype(np.float32)
    if _debug:
        kernel.debug_results = res
    if _trace:
        kernel.trace_results = res
    return out


# revision 13
# speedup vs baseline: 1.3889x; 1.0311x over previous
"""Trainium2 Bass kernel for BoringFeedForwardMOE (E=8 experts, top-2, cap=2048).

Strategy: expert parallelism across 8 NeuronCores.
  - Router computed data-parallel (each core: logits for its 1024-token slice,
    f32 to match reference top-k decisions bit-for-bit), AllGather of logits.
  - Full routing (top-2 + softmax gates + capacity ranks) recomputed identically
    on every core from the gathered [8192, 8] logits; prefix sums over tokens
    via log-step shifted adds (free axis) + strictly-upper-triangular matmul
    (partition axis).
  - Dispatch: one batched dma_scatter_add builds buf (slot -> token id) in
    DRAM (unselected/over-capacity entries land in a dump row); the FFN input
    is then produced chunk-by-chunk with dma_gather(transpose=True), which
    reads each slot's bf16 x-row from HBM directly into [feature, slot]
    layout for the TensorEngine.
  - FFN in bf16, gelu + biases fused. Outputs are AllGathered in 4 chunks of
    512 slots so the collective overlaps the FFN of later chunks.
  - Combine: two batched dma_gathers fetch each own token's <=2 expert
    contributions (dropped ones hit a zeroed pad row), f32 gates, write out.

Token layout on device: t = p*64 + n (p: partition 0..127, n: 0..63).
Core c owns tokens [c*1024, (c+1)*1024) and expert e = c.
agout row for contribution (e, r): (r>>8)*2048 + e*256 + (r & 255).
"""
import sys

if "/opt/trn_rl_repo" not in sys.path:
    sys.path.insert(0, "/opt/trn_rl_repo")

import numpy as np
import ml_dtypes

import concourse.bass as bass
import concourse.bacc as bacc
import concourse.mybir as mybir
from concourse.tile import TileContext
from concourse.bass_utils import run_bass_kernel_spmd

F32 = mybir.dt.float32
BF16 = mybir.dt.bfloat16
I32 = mybir.dt.int32
AF = mybir.ActivationFunctionType
OP = mybir.AluOpType
AX = mybir.AxisListType

NCORES = 8
P = 128
NTOK = 8192          # B*T
C = 1024
HD = 4096
E = 8
CAP = 2048
NT = NTOK // P       # 64 token columns per partition
BIG = 1.0e6
TCH = 256            # FFN slot chunk (per PSUM group)
NCH = CAP // TCH     # 8 chunks
AGCH = 256           # slots per output-AllGather chunk
NAG = CAP // AGCH    # 8 collectives

_BUILT = None


def _r3(ap, e=E):
    return ap.rearrange("p (n e) -> p n e", e=e)


def build(debug=False):
    nc = bacc.Bacc()

    # ---- per-core parameters -------------------------------------------------
    xt = nc.declare_dram_parameter("xt", [C, 1024], F32, isOutput=False)
    xb = nc.declare_dram_parameter("xb", [NTOK, C], BF16, isOutput=False)
    wr = nc.declare_dram_parameter("wr", [P, 64], F32, isOutput=False)
    brr = nc.declare_dram_parameter("brr", [1, E], F32, isOutput=False)
    w1 = nc.declare_dram_parameter("w1", [C, HD], BF16, isOutput=False)
    b1a = nc.declare_dram_parameter("b1a", [P, HD // P], F32, isOutput=False)
    w2 = nc.declare_dram_parameter("w2", [HD, C], BF16, isOutput=False)
    b2r = nc.declare_dram_parameter("b2r", [1, C], F32, isOutput=False)
    tri = nc.declare_dram_parameter("tri", [P, P], F32, isOutput=False)
    binv = nc.declare_dram_parameter("binv", [P, NT * E], F32, isOutput=False)
    cmask = nc.declare_dram_parameter("cmask", [P, NT * E], F32, isOutput=False)
    ones1 = nc.declare_dram_parameter("ones1", [1, P], F32, isOutput=False)
    cidp = nc.declare_dram_parameter("cidp", [P, 1], I32, isOutput=False)
    tokid = nc.declare_dram_parameter("tokid", [P, NT], mybir.dt.int16,
                                      isOutput=False)
    dmpr = nc.declare_dram_parameter("dmpr", [P, NT], F32, isOutput=False)
    out = nc.declare_dram_parameter("out", [1024, C], F32, isOutput=True)
    if debug:
        dbg_lg = nc.declare_dram_parameter("dbg_lg", [NTOK, E], F32, isOutput=True)
        dbg_rs = nc.declare_dram_parameter("dbg_rs", [1024, 32], F32, isOutput=True)
        dbg_bf = nc.declare_dram_parameter("dbg_bf", [CAP + P, P], F32, isOutput=True)
        dbg_ag = nc.declare_dram_parameter("dbg_ag", [E * CAP, C], F32, isOutput=True)

    # ---- internal DRAM -------------------------------------------------------
    lgin = nc.dram_tensor("lgin", [1024, E], F32)
    lgout = nc.dram_tensor("lgout", [NTOK, E], F32, addr_space="Shared")
    # buf_d row s (first int16) = token id at slot s of this expert; row CAP is
    # the dump row for unselected / over-capacity entries.
    buf_ds = [nc.dram_tensor(f"buf_d{i}", [CAP + P, P], mybir.dt.int16)
              for i in range(4)]
    xg = nc.dram_tensor("xg", [CAP, C], BF16)
    didx_d = nc.dram_tensor("didx_d", [NTOK], mybir.dt.int16)
    cscr = nc.dram_tensor("cscr", [2, 1024], mybir.dt.int16)
    agin = nc.dram_tensor("agin", [CAP, C], BF16)
    # chunk k of agin ([AGCH, C] slots) -> agout rows [k*E*AGCH, (k+1)*E*AGCH);
    # rows [E*CAP, E*CAP+P) are a zeroed pad target for dropped contributions.
    agout = nc.dram_tensor("agout", [E * CAP + P, C], BF16, addr_space="Shared")
    rsc = nc.dram_tensor("rsc", [NTOK // 8, 32], F32)   # packed idx1/idx2/g1/g2

    rg = [list(range(NCORES))]

    with TileContext(nc) as tc:
        with tc.tile_pool(name="wpool", bufs=1) as wp:
            on1 = wp.tile([1, P], F32, tag="ones1")
            nc.sync.dma_start(out=on1[:], in_=ones1[:])
            cis = wp.tile([P, 1], I32, tag="cidp")
            nc.sync.dma_start(out=cis[:], in_=cidp[:])
            tks = wp.tile([P, NT], mybir.dt.int16, tag="tokid")
            nc.sync.dma_start(out=tks[:], in_=tokid[:])
            dms = wp.tile([P, NT], F32, tag="dmpr")
            nc.sync.dma_start(out=dms[:], in_=dmpr[:])
            # zero-fill buf_d (scatter-add target) and the agout pad rows
            zi = wp.tile([P, P], mybir.dt.int16, tag="zi")
            nc.vector.memset(zi[:], 0)
            for b in buf_ds:
                for i in range((CAP + P) // P):
                    nc.sync.dma_start(out=b[i * P:(i + 1) * P, :], in_=zi[:])
            zb = wp.tile([P, C], BF16, tag="zb")
            nc.vector.memset(zb[:], 0.0)
            nc.sync.dma_start(out=agout[E * CAP:E * CAP + P, :], in_=zb[:])

            # ================= Phase A: local router logits =================
            with (
                tc.tile_pool(name="rpool", bufs=2) as rp,
                tc.tile_pool(name="rps", bufs=2, space="PSUM") as rps,
            ):
                wrs = rp.tile([P, 64], F32, tag="wr")
                nc.sync.dma_start(out=wrs[:], in_=wr[:])
                brs = rp.tile([1, E], F32, tag="brr")
                nc.sync.dma_start(out=brs[:], in_=brr[:])
                xts = []
                for k in range(C // P):
                    t = rp.tile([P, 1024], F32, tag=f"xt_{k}")
                    nc.sync.dma_start(out=t[:], in_=xt[k * P:(k + 1) * P, :])
                    xts.append(t)
                for m in range(1024 // P):
                    ps = rps.tile([P, E], F32, tag="lg")
                    for k in range(C // P):
                        nc.tensor.matmul(
                            ps[:], lhsT=xts[k][:, m * P:(m + 1) * P],
                            rhs=wrs[:, k * E:(k + 1) * E],
                            start=(k == 0), stop=False)
                    nc.tensor.matmul(ps[:], lhsT=on1[:], rhs=brs[:],
                                     start=False, stop=True)
                    lgs = rp.tile([P, E], F32, tag="lgout")
                    nc.scalar.activation(lgs[:], ps[:], AF.Copy)
                    nc.sync.dma_start(out=lgin[m * P:(m + 1) * P, :], in_=lgs[:])

            nc.gpsimd.collective_compute(
                "AllGather", OP.bypass, ins=[lgin[:]], outs=[lgout[:]],
                replica_groups=rg)
            if debug:
                nc.sync.dma_start(out=dbg_lg[:], in_=lgout[:])

            # ================= Phase B: full routing ========================
            with (
                tc.tile_pool(name="bpool", bufs=1) as bp,
                tc.tile_pool(name="bps", bufs=2, space="PSUM") as bps,
            ):
                W = NT * E  # 512
                L = bp.tile([P, W], F32, tag="L")
                nc.sync.dma_start(
                    out=_r3(L[:]),
                    in_=lgout[:].rearrange("(p n) e -> p n e", p=P))
                bv = bp.tile([P, W], F32, tag="binv")
                nc.sync.dma_start(out=bv[:], in_=binv[:])
                cm = bp.tile([P, W], F32, tag="cmask")
                nc.sync.dma_start(out=cm[:], in_=cmask[:])
                trs = bp.tile([P, P], F32, tag="tri")
                nc.sync.dma_start(out=trs[:], in_=tri[:])

                def tt(o, a, b, op):
                    nc.vector.tensor_tensor(out=o, in0=a, in1=b, op=op)

                v1 = bp.tile([P, NT], F32, tag="v1")
                nc.vector.reduce_max(v1[:], _r3(L[:]), axis=AX.X)
                m1 = bp.tile([P, W], F32, tag="t0")
                tt(_r3(m1[:]), _r3(L[:]), v1[:].to_broadcast([P, NT, E]),
                   OP.is_equal)
                tmp = bp.tile([P, W], F32, tag="t1")
                tt(tmp[:], m1[:], bv[:], OP.mult)
                e1x = bp.tile([P, NT], F32, tag="e1x")
                nc.vector.reduce_max(e1x[:], _r3(tmp[:]), axis=AX.X)
                oh1 = bp.tile([P, W], F32, tag="oh1")
                tt(_r3(oh1[:]), _r3(bv[:]), e1x[:].to_broadcast([P, NT, E]),
                   OP.is_equal)
                msk = bp.tile([P, W], F32, tag="t0b")
                nc.vector.tensor_scalar(out=msk[:], in0=oh1[:], scalar1=-BIG,
                                        scalar2=None, op0=OP.mult)
                tt(msk[:], L[:], msk[:], OP.add)
                v2 = bp.tile([P, NT], F32, tag="v2")
                nc.vector.reduce_max(v2[:], _r3(msk[:]), axis=AX.X)
                m2 = bp.tile([P, W], F32, tag="t2")
                tt(_r3(m2[:]), _r3(msk[:]), v2[:].to_broadcast([P, NT, E]),
                   OP.is_equal)
                tt(m2[:], m2[:], bv[:], OP.mult)
                e2x = bp.tile([P, NT], F32, tag="e2x")
                nc.vector.reduce_max(e2x[:], _r3(m2[:]), axis=AX.X)
                oh2 = bp.tile([P, W], F32, tag="oh2")
                tt(_r3(oh2[:]), _r3(bv[:]), e2x[:].to_broadcast([P, NT, E]),
                   OP.is_equal)
                sel = bp.tile([P, W], F32, tag="sel")
                tt(sel[:], oh1[:], oh2[:], OP.add)

                vd = bp.tile([P, NT], F32, tag="vd")
                tt(vd[:], v1[:], v2[:], OP.subtract)
                g1 = bp.tile([P, NT], F32, tag="g1")
                nc.scalar.activation(g1[:], vd[:], AF.Sigmoid)
                g2 = bp.tile([P, NT], F32, tag="g2")
                nc.vector.tensor_scalar(out=g2[:], in0=g1[:], scalar1=-1.0,
                                        scalar2=1.0, op0=OP.mult, op1=OP.add)

                # inclusive prefix over n (shift s tokens == 8s columns)
                cur = sel
                pidx = 0
                for s in (1, 2, 4, 8, 16, 32):
                    nxt = bp.tile([P, W], F32, tag=f"pf{pidx % 2}")
                    pidx += 1
                    tt(nxt[:, 8 * s:], cur[:, 8 * s:], cur[:, :W - 8 * s], OP.add)
                    nc.vector.tensor_copy(out=nxt[:, :8 * s], in_=cur[:, :8 * s])
                    cur = nxt
                incl = cur
                offp = bps.tile([P, E], F32, tag="offp")
                nc.tensor.matmul(offp[:], lhsT=trs[:], rhs=incl[:, W - E:W],
                                 start=True, stop=True)
                offs = bp.tile([P, E], F32, tag="offs")
                nc.scalar.activation(offs[:], offp[:], AF.Copy)
                rank = bp.tile([P, W], F32, tag="rank")
                tt(rank[:], incl[:], sel[:], OP.subtract)
                offs3 = bass.AP(
                    offs[:].tensor, offs[:].offset,
                    [offs[:].ap[0], [0, NT], offs[:].ap[1]])
                tt(_r3(rank[:]), _r3(rank[:]), offs3, OP.add)

                # dispatch indices for my expert: disp = rank + (1-sel)*BIG,
                # clamped to the dump row CAP, as int16 token-major
                disp = bp.tile([P, W], F32, tag="disp")
                nc.vector.tensor_scalar(out=disp[:], in0=sel[:], scalar1=-BIG,
                                        scalar2=BIG, op0=OP.mult, op1=OP.add)
                tt(disp[:], rank[:], disp[:], OP.add)
                dcf = bp.tile([P, W], F32, tag="dcf")
                tt(dcf[:], disp[:], cm[:], OP.mult)
                dce = bp.tile([P, NT], F32, tag="dce")
                nc.vector.reduce_sum(dce[:], _r3(dcf[:]), axis=AX.X)
                # clamp dropped entries onto per-token-spread dump rows so the
                # scatter-add's CCE RMW chains don't hammer one HBM row
                dcc = bp.tile([P, NT], F32, tag="dcc")
                tt(dcc[:], dce[:], dms[:], OP.min)
                dci = bp.tile([P, NT], mybir.dt.int16, tag="dci")
                nc.vector.tensor_copy(out=dci[:], in_=dcc[:])
                # bounce token-major to DRAM, reload in the 16-partition
                # index-wrap layout the batched SWDGE ops expect, replicate to
                # all 8 Q7 core blocks
                nc.sync.dma_start(
                    out=didx_d[:].rearrange("(p n) -> p n", p=P), in_=dci[:])
                didx = wp.tile([P, W], mybir.dt.int16, tag="didx")
                nc.sync.dma_start(
                    out=didx[0:16, :].rearrange("q (n b) -> q n b", b=8),
                    in_=didx_d[:].rearrange("(b q n) -> q n b", q=16, n=NT))
                for r in range(1, 8):
                    nc.sync.dma_start(out=didx[16 * r:16 * (r + 1), :],
                                      in_=didx[0:16, :])

                # combine indices, chunk-strided AG layout:
                # idx = (r>>8)*2048 + e*256 + (r&255)  (+BIG if dropped/unsel)
                def mkidx(ohx, exx, tag):
                    ei = bp.tile([P, NT], F32, tag=f"ei{tag}")
                    nc.vector.tensor_scalar(out=ei[:], in0=exx[:], scalar1=-1.0,
                                            scalar2=BIG, op0=OP.mult, op1=OP.add)
                    tmpr = bp.tile([P, W], F32, tag="t3")
                    tt(tmpr[:], rank[:], ohx[:], OP.mult)
                    ri = bp.tile([P, NT], F32, tag=f"ri{tag}")
                    nc.vector.reduce_sum(ri[:], _r3(tmpr[:]), axis=AX.X)
                    rii = bp.tile([P, NT], I32, tag=f"rii{tag}")
                    nc.vector.tensor_copy(out=rii[:], in_=ri[:])
                    rc = bp.tile([P, NT], I32, tag=f"rc{tag}")
                    nc.vector.tensor_scalar(out=rc[:], in0=rii[:], scalar1=8,
                                            scalar2=11, op0=OP.arith_shift_right,
                                            op1=OP.logical_shift_left)
                    rl = bp.tile([P, NT], I32, tag=f"rl{tag}")
                    nc.vector.tensor_scalar(out=rl[:], in0=rii[:], scalar1=255,
                                            scalar2=None, op0=OP.bitwise_and)
                    tt(rc[:], rc[:], rl[:], OP.add)
                    # dropped: r >= CAP -> +BIG
                    di = bp.tile([P, NT], F32, tag=f"di{tag}")
                    nc.vector.tensor_scalar(out=di[:], in0=ri[:],
                                            scalar1=float(CAP) - 0.5,
                                            scalar2=BIG,
                                            op0=OP.is_gt, op1=OP.mult)
                    ix = bp.tile([P, NT], F32, tag=f"ix{tag}")
                    nc.vector.tensor_scalar(out=ix[:], in0=ei[:],
                                            scalar1=256.0, scalar2=None,
                                            op0=OP.mult)
                    tt(ix[:], ix[:], di[:], OP.add)
                    rcf = bp.tile([P, NT], F32, tag=f"rcf{tag}")
                    nc.vector.tensor_copy(out=rcf[:], in_=rc[:])
                    tt(ix[:], ix[:], rcf[:], OP.add)
                    # dropped contributions point at the zeroed pad row
                    nc.vector.tensor_scalar(out=ix[:], in0=ix[:],
                                            scalar1=float(E * CAP),
                                            scalar2=None, op0=OP.min)
                    return ix

                ix1 = mkidx(oh1, e1x, "1")
                ix2 = mkidx(oh2, e2x, "2")

                pk = bp.tile([P, NT * 4], F32, tag="pk")
                pk4 = pk[:].rearrange("p (n f) -> p n f", f=4)
                nc.vector.tensor_copy(out=pk4[:, :, 0], in_=ix1[:])
                nc.vector.tensor_copy(out=pk4[:, :, 1], in_=ix2[:])
                nc.vector.tensor_copy(out=pk4[:, :, 2], in_=g1[:])
                nc.vector.tensor_copy(out=pk4[:, :, 3], in_=g2[:])
                nc.sync.dma_start(
                    out=rsc[:].rearrange("(p q) f -> p q f", p=P), in_=pk4)
                if debug:
                    nc.sync.dma_start(out=dbg_rs[:], in_=rsc[:])

                # ============== Phase C: dispatch scatter ===================
                # own-token combine metadata prefetch (independent of AGs)
                own = wp.tile([P, 32], F32, tag="own")
                nc.gpsimd.indirect_dma_start(
                    out=own[:], out_offset=None,
                    in_=rsc[:],
                    in_offset=bass.IndirectOffsetOnAxis(ap=cis[:, :1], axis=0),
                )
                own4 = own[:].rearrange("p (n f) -> p n f", f=4)
                i1 = wp.tile([P, 8], mybir.dt.int16, tag="i1")
                nc.vector.tensor_copy(out=i1[:], in_=own4[:, :, 0])
                i2 = wp.tile([P, 8], mybir.dt.int16, tag="i2")
                nc.vector.tensor_copy(out=i2[:], in_=own4[:, :, 1])
                go1 = wp.tile([P, 8], F32, tag="go1")
                nc.vector.tensor_copy(out=go1[:], in_=own4[:, :, 2])
                go2 = wp.tile([P, 8], F32, tag="go2")
                nc.vector.tensor_copy(out=go2[:], in_=own4[:, :, 3])
                # combine-gather index tiles: entry i = p' + 128*n' for own
                # token j = p'*8 + n', in 16-partition wrap, replicated
                cidx = []
                for f, it in ((0, i1), (1, i2)):
                    nc.sync.dma_start(
                        out=cscr[f, :].rearrange("(p n) -> p n", p=P), in_=it[:])
                    ci = wp.tile([P, 64], mybir.dt.int16, tag=f"cidx{f}")
                    nc.sync.dma_start(
                        out=ci[0:16, :].rearrange("q (n b) -> q n b", b=8),
                        in_=cscr[f, :].rearrange("(b q n) -> q n b", q=16, n=8))
                    for r in range(1, 8):
                        nc.sync.dma_start(out=ci[16 * r:16 * (r + 1), :],
                                          in_=ci[0:16, :])
                    cidx.append(ci)

                # scatter token ids into buf_d[slot] (dump row absorbs the
                # rest); scatter-add requires 256B elements, so each id is
                # replicated across a 128-wide int16 row
                tkb = bp.tile([P, NT * P], mybir.dt.int16, tag="tkb")
                nc.vector.tensor_copy(
                    out=tkb[:].rearrange("p (n e) -> p n e", e=P),
                    in_=tks[:].to_broadcast([P, NT, P]))
                tkb3 = tkb[:].rearrange("p (n e) -> p n e", e=P)
                for k in range(NTOK // 512):
                    nc.gpsimd.dma_scatter_add(
                        out_ap=buf_ds[k % 4][:],
                        in_ap=tkb3[:, k * 4:(k + 1) * 4, :],
                        idxs_ap=didx[:, k * 32:(k + 1) * 32],
                        num_idxs=512,
                        num_idxs_reg=512,
                        elem_size=P,
                        single_packet=False,
                    )
                # reload the four buf shards (each slot written by exactly one
                # scatter chain; the rest hold zeros) and sum them
                bufi = wp.tile([P, CAP // 16], mybir.dt.int16, tag="bufi")
                parts = []
                for i in range(4):
                    t = wp.tile([16, CAP // 16], mybir.dt.int16, tag=f"bfp{i}")
                    nc.sync.dma_start(
                        out=t[:],
                        in_=buf_ds[i][0:CAP, 0:1].rearrange(
                            "(s q) one -> q (s one)", q=16))
                    parts.append(t)
                nc.vector.tensor_tensor(out=parts[0][:], in0=parts[0][:],
                                        in1=parts[1][:], op=OP.add)
                nc.vector.tensor_tensor(out=parts[2][:], in0=parts[2][:],
                                        in1=parts[3][:], op=OP.add)
                nc.vector.tensor_tensor(out=bufi[0:16, :], in0=parts[0][:],
                                        in1=parts[2][:], op=OP.add)
                for r in range(1, 8):
                    nc.sync.dma_start(out=bufi[16 * r:16 * (r + 1), :],
                                      in_=bufi[0:16, :])

            # ---- weights (loaded behind router/dispatch in priority order) --
            w1t = []
            for k in range(C // P):
                t = wp.tile([P, HD], BF16, tag=f"w1_{k}")
                nc.sync.dma_start(out=t[:], in_=w1[k * P:(k + 1) * P, :])
                w1t.append(t)
            w2t = []
            for k in range(HD // P):
                t = wp.tile([P, C], BF16, tag=f"w2_{k}")
                nc.sync.dma_start(out=t[:], in_=w2[k * P:(k + 1) * P, :])
                w2t.append(t)
            b1s = wp.tile([P, HD // P], F32, tag="b1a")
            nc.sync.dma_start(out=b1s[:], in_=b1a[:])
            b2s = wp.tile([1, C], F32, tag="b2r")
            nc.sync.dma_start(out=b2s[:], in_=b2r[:])

            # ================= Phase D: expert FFN ==========================
            with (
                tc.tile_pool(name="fpool", bufs=1) as fp,
                tc.tile_pool(name="fps", bufs=4, space="PSUM") as fpsH,
                tc.tile_pool(name="fps2", bufs=2, space="PSUM") as fpsM,
                tc.tile_pool(name="xgt", bufs=2) as xp,
                tc.tile_pool(name="osb", bufs=3) as op_,
            ):
                for chk in range(NCH):
                    gout = xp.tile([P, (TCH // P) * C], BF16, tag="gout")
                    gout3 = gout[:].rearrange("p (a c) -> p a c", c=C)
                    nc.gpsimd.dma_gather(
                        out_ap=gout3,
                        in_ap=xb[:],
                        idxs_ap=bufi[:, chk * (TCH // 16):(chk + 1) * (TCH // 16)],
                        num_idxs=TCH,
                        num_idxs_reg=TCH,
                        elem_size=C,
                        transpose=False,
                        single_packet=False,
                    )
                    nc.sync.dma_start(
                        out=xg[chk * TCH:(chk + 1) * TCH, :].rearrange(
                            "(a p) c -> p a c", p=P),
                        in_=gout3)
                    xgt = xp.tile([P, (C // P) * TCH], BF16, tag="xgt")
                    xgt3 = xgt[:].rearrange("p (k s) -> p k s", s=TCH)
                    for k in range(C // P):
                        nc.scalar.dma_start(
                            out=xgt3[:, k, :],
                            in_=xg[chk * TCH:(chk + 1) * TCH, k * P:(k + 1) * P],
                            transpose=True)
                    hs = []
                    for m in range(HD // P):
                        ps = fpsH.tile([P, TCH], F32, tag="hps")
                        for k in range(C // P):
                            nc.tensor.matmul(
                                ps[:], lhsT=w1t[k][:, m * P:(m + 1) * P],
                                rhs=xgt3[:, k, :], start=(k == 0),
                                stop=(k == C // P - 1))
                        h = fp.tile([P, TCH], BF16, tag=f"h_{chk % 2}_{m}")
                        nc.scalar.activation(h[:], ps[:], AF.Gelu,
                                             bias=b1s[:, m:m + 1])
                        hs.append(h)
                    for st in range(TCH // P):
                        for cf in range(2):
                            ps2 = fpsM.tile([P, 512], F32, tag="ops")
                            for k in range(HD // P):
                                nc.tensor.matmul(
                                    ps2[:], lhsT=hs[k][:, st * P:(st + 1) * P],
                                    rhs=w2t[k][:, cf * 512:(cf + 1) * 512],
                                    start=(k == 0), stop=False)
                            nc.tensor.matmul(
                                ps2[:], lhsT=on1[:],
                                rhs=b2s[:, cf * 512:(cf + 1) * 512],
                                start=False, stop=True)
                            ob = op_.tile([P, 512], BF16, tag="ob")
                            nc.scalar.activation(ob[:], ps2[:], AF.Copy)
                            nc.sync.dma_start(
                                out=agin[chk * TCH + st * P:
                                         chk * TCH + (st + 1) * P,
                                         cf * 512:(cf + 1) * 512],
                                in_=ob[:])
                    # ---- chunked output AllGather (overlaps later chunks) ---
                    if (chk + 1) % (AGCH // TCH) == 0:
                        ag = (chk + 1) // (AGCH // TCH) - 1
                        nc.gpsimd.collective_compute(
                            "AllGather", OP.bypass,
                            ins=[agin[ag * AGCH:(ag + 1) * AGCH, :]],
                            outs=[agout[ag * E * AGCH:(ag + 1) * E * AGCH, :]],
                            replica_groups=rg)

            if debug:
                dbx = wp.tile([P, C], F32, tag="dbx")
                dbb = wp.tile([P, P], mybir.dt.int16, tag="dbb")
                dbf = wp.tile([P, P], F32, tag="dbf")
                for i in range((CAP + P) // P):
                    nc.sync.dma_start(out=dbb[:], in_=buf_d[i * P:(i + 1) * P, :])
                    nc.vector.tensor_copy(out=dbf[:], in_=dbb[:])
                    nc.sync.dma_start(out=dbg_bf[i * P:(i + 1) * P, :], in_=dbf[:])
                for i in range(E * CAP // P):
                    nc.sync.dma_start(out=dbx[:], in_=agout[i * P:(i + 1) * P, :])
                    nc.sync.dma_start(out=dbg_ag[i * P:(i + 1) * P, :], in_=dbx[:])

            # ================= Phase F: combine own tokens ==================
            with tc.tile_pool(name="cpool", bufs=1) as cp:
                cg = []
                for f in range(2):
                    halves = []
                    for h in range(2):
                        t = cp.tile([P, 4 * C], BF16, tag=f"cg{f}_{h}")
                        nc.gpsimd.dma_gather(
                            out_ap=t[:].rearrange("p (n c) -> p n c", c=C),
                            in_ap=agout[:],
                            idxs_ap=cidx[f][:, h * 32:(h + 1) * 32],
                            num_idxs=512,
                            num_idxs_reg=512,
                            elem_size=C,
                            transpose=False,
                            single_packet=False,
                        )
                        halves.append(t[:].rearrange("p (n c) -> p n c", c=C))
                    cg.append(halves)
                with tc.tile_pool(name="copool", bufs=3) as cop:
                    for j in range(8):
                        h, jj = j // 4, j % 4
                        o1 = cop.tile([P, C], F32, tag="o1")
                        nc.vector.tensor_scalar(out=o1[:], in0=cg[0][h][:, jj, :],
                                                scalar1=go1[:, j:j + 1],
                                                scalar2=None, op0=OP.mult)
                        o2 = cop.tile([P, C], F32, tag="o2")
                        nc.vector.tensor_scalar(out=o2[:], in0=cg[1][h][:, jj, :],
                                                scalar1=go2[:, j:j + 1],
                                                scalar2=None, op0=OP.mult)
                        nc.vector.tensor_tensor(out=o1[:], in0=o1[:], in1=o2[:],
                                                op=OP.add)
                        nc.sync.dma_start(
                            out=out[:].rearrange("(p n) c -> p n c", p=P)[:, j, :],
                            in_=o1[:])

    nc.compile()
    return nc


def _host_inputs(x, Wr, br, W1, b1, W2, b2):
    xf = np.ascontiguousarray(x.reshape(NTOK, C)).astype(np.float32)
    xb = xf.astype(ml_dtypes.bfloat16)
    wr_a = np.ascontiguousarray(
        Wr.reshape(C // P, P, E).transpose(1, 0, 2).reshape(P, 64)).astype(
            np.float32)
    brr = br.reshape(1, E).astype(np.float32)
    tri = np.triu(np.ones((P, P), np.float32), 1)
    binv = np.broadcast_to(
        np.tile(BIG - np.arange(E, dtype=np.float32), NT), (P, NT * E)).copy()
    ones1 = np.ones((1, P), np.float32)
    tokid = (np.arange(P, dtype=np.int32)[:, None] * NT
             + np.arange(NT, dtype=np.int32)[None, :]).astype(np.int16)
    dmpr = (CAP + (np.arange(P)[:, None] * NT + np.arange(NT)[None, :]) % P
            ).astype(np.float32)
    in_maps = []
    for c in range(NCORES):
        cm = np.zeros(E, np.float32)
        cm[c] = 1.0
        cmask = np.broadcast_to(np.tile(cm, NT), (P, NT * E)).copy()
        in_maps.append({
            "xt": np.ascontiguousarray(xf[c * 1024:(c + 1) * 1024, :].T),
            "xb": xb,
            "wr": wr_a,
            "brr": brr,
            "w1": np.ascontiguousarray(W1[c]).astype(ml_dtypes.bfloat16),
            "b1a": np.ascontiguousarray(
                b1[c].reshape(HD // P, P).T).astype(np.float32),
            "w2": np.ascontiguousarray(W2[c]).astype(ml_dtypes.bfloat16),
            "b2r": b2[c].reshape(1, C).astype(np.float32),
            "tri": tri,
            "binv": binv,
            "cmask": cmask,
            "ones1": ones1,
            "cidp": (c * P + np.arange(P, dtype=np.int32)).reshape(P, 1),
            "tokid": tokid,
            "dmpr": dmpr,
        })
    return in_maps


def _ensure_ntff_hook():
    """The agent image's antenv lacks axon_hooks; shim it so trace=True works."""
    import types
    try:
        import antenv.axon_hooks  # noqa: F401
        return
    except ImportError:
        pass
    import antenv
    mod = types.ModuleType("antenv.axon_hooks")
    state = {"h": None}
    mod.set_axon_ntff_profile_hook = lambda h: state.__setitem__("h", h)
    mod.get_axon_ntff_profile_hook = lambda: state["h"]
    sys.modules["antenv.axon_hooks"] = mod
    antenv.axon_hooks = mod
    from trn_agent_boot.trn_boot import _ntff_profile_via_ctypes
    mod.set_axon_ntff_profile_hook(
        _ntff_profile_via_ctypes("/opt/axon/libaxon_pjrt.so"))


def kernel(x, Wr, br, W1, b1, W2, b2, _debug=False, _trace=False):
    global _BUILT
    x, Wr, br = np.asarray(x), np.asarray(Wr), np.asarray(br)
    W1, b1, W2, b2 = map(np.asarray, (W1, b1, W2, b2))
    if _BUILT is None or _BUILT[1] != _debug:
        _BUILT = (build(debug=_debug), _debug)
    nc = _BUILT[0]
    in_maps = _host_inputs(x, Wr, br, W1, b1, W2, b2)
    if _trace:
        _ensure_ntff_hook()
    res = run_bass_kernel_spmd(nc, in_maps, list(range(NCORES)), trace=_trace)
    outs = np.concatenate([res.results[c]["out"] for c in range(NCORES)], 0)
    out = outs.reshape(x.shape).astype(np.float32)
    if _debug:
        kernel.debug_results = res
    if _trace:
        kernel.trace_results = res
    return out


# revision 17
# speedup vs baseline: 1.4222x; 1.0240x over previous
"""Trainium2 Bass kernel for BoringFeedForwardMOE (E=8 experts, top-2, cap=2048).

Strategy: expert parallelism across 8 NeuronCores.
  - Router computed data-parallel (each core: logits for its 1024-token slice,
    f32 to match reference top-k decisions bit-for-bit), AllGather of logits.
  - Full routing (top-2 + softmax gates + capacity ranks) recomputed identically
    on every core from the gathered [8192, 8] logits; prefix sums over tokens
    via log-step shifted adds (free axis) + strictly-upper-triangular matmul
    (partition axis).
  - Dispatch: one batched dma_scatter_add builds buf (slot -> token id) in
    DRAM (unselected/over-capacity entries land in a dump row); the FFN input
    is then produced chunk-by-chunk with dma_gather(transpose=True), which
    reads each slot's bf16 x-row from HBM directly into [feature, slot]
    layout for the TensorEngine.
  - FFN in bf16, gelu + biases fused. Outputs are AllGathered in 4 chunks of
    512 slots so the collective overlaps the FFN of later chunks.
  - Combine: two batched dma_gathers fetch each own token's <=2 expert
    contributions (dropped ones hit a zeroed pad row), f32 gates, write out.

Token layout on device: t = p*64 + n (p: partition 0..127, n: 0..63).
Core c owns tokens [c*1024, (c+1)*1024) and expert e = c.
agout row for contribution (e, r): (r>>8)*2048 + e*256 + (r & 255).
"""
import sys

if "/opt/trn_rl_repo" not in sys.path:
    sys.path.insert(0, "/opt/trn_rl_repo")

import numpy as np
import ml_dtypes

import concourse.bass as bass
import concourse.bacc as bacc
import concourse.mybir as mybir
from concourse.tile import TileContext
from concourse.bass_utils import run_bass_kernel_spmd

F32 = mybir.dt.float32
BF16 = mybir.dt.bfloat16
I32 = mybir.dt.int32
AF = mybir.ActivationFunctionType
OP = mybir.AluOpType
AX = mybir.AxisListType

NCORES = 8
P = 128
NTOK = 8192          # B*T
C = 1024
HD = 4096
E = 8
CAP = 2048
NT = NTOK // P       # 64 token columns per partition
BIG = 1.0e6
TCH = 256            # FFN slot chunk (per PSUM group)
NCH = CAP // TCH     # 8 chunks
AGCH = 256           # slots per output-AllGather chunk
NAG = CAP // AGCH    # 8 collectives

_BUILT = None


def _r3(ap, e=E):
    return ap.rearrange("p (n e) -> p n e", e=e)


def build(debug=False):
    nc = bacc.Bacc()

    # ---- per-core parameters -------------------------------------------------
    xt = nc.declare_dram_parameter("xt", [C, 1024], F32, isOutput=False)
    xb = nc.declare_dram_parameter("xb", [NTOK, C], BF16, isOutput=False)
    wr = nc.declare_dram_parameter("wr", [P, 64], F32, isOutput=False)
    brr = nc.declare_dram_parameter("brr", [1, E], F32, isOutput=False)
    w1 = nc.declare_dram_parameter("w1", [C, HD], BF16, isOutput=False)
    b1a = nc.declare_dram_parameter("b1a", [P, HD // P], F32, isOutput=False)
    w2 = nc.declare_dram_parameter("w2", [HD, C], BF16, isOutput=False)
    b2r = nc.declare_dram_parameter("b2r", [1, C], F32, isOutput=False)
    tri = nc.declare_dram_parameter("tri", [P, P], F32, isOutput=False)
    binv = nc.declare_dram_parameter("binv", [P, NT * E], F32, isOutput=False)
    cmask = nc.declare_dram_parameter("cmask", [P, NT * E], F32, isOutput=False)
    ones1 = nc.declare_dram_parameter("ones1", [1, P], F32, isOutput=False)
    cidp = nc.declare_dram_parameter("cidp", [P, 1], I32, isOutput=False)
    tokid = nc.declare_dram_parameter("tokid", [P, NT], mybir.dt.int16,
                                      isOutput=False)
    dmpr = nc.declare_dram_parameter("dmpr", [P, NT], F32, isOutput=False)
    out = nc.declare_dram_parameter("out", [1024, C], F32, isOutput=True)
    if debug:
        dbg_lg = nc.declare_dram_parameter("dbg_lg", [NTOK, E], F32, isOutput=True)
        dbg_rs = nc.declare_dram_parameter("dbg_rs", [1024, 32], F32, isOutput=True)
        dbg_bf = nc.declare_dram_parameter("dbg_bf", [CAP + P, P], F32, isOutput=True)
        dbg_ag = nc.declare_dram_parameter("dbg_ag", [E * CAP, C], F32, isOutput=True)

    # ---- internal DRAM -------------------------------------------------------
    lgin = nc.dram_tensor("lgin", [1024, E], F32)
    lgout = nc.dram_tensor("lgout", [NTOK, E], F32, addr_space="Shared")
    # buf_d row s (first int16) = token id at slot s of this expert; row CAP is
    # the dump row for unselected / over-capacity entries.
    buf_ds = [nc.dram_tensor(f"buf_d{i}", [CAP + P, P], mybir.dt.int16)
              for i in range(4)]
    xg = nc.dram_tensor("xg", [CAP, C], BF16)
    didx_d = nc.dram_tensor("didx_d", [NTOK], mybir.dt.int16)
    cscr = nc.dram_tensor("cscr", [2, 1024], mybir.dt.int16)
    agin = nc.dram_tensor("agin", [CAP, C], BF16)
    # chunk k of agin ([AGCH, C] slots) -> agout rows [k*E*AGCH, (k+1)*E*AGCH);
    # rows [E*CAP, E*CAP+P) are a zeroed pad target for dropped contributions.
    agout = nc.dram_tensor("agout", [E * CAP + P, C], BF16, addr_space="Shared")
    rsc = nc.dram_tensor("rsc", [NTOK // 8, 32], F32)   # packed idx1/idx2/g1/g2

    rg = [list(range(NCORES))]

    with TileContext(nc) as tc:
        with tc.tile_pool(name="wpool", bufs=1) as wp:
            on1 = wp.tile([1, P], F32, tag="ones1")
            nc.sync.dma_start(out=on1[:], in_=ones1[:])
            cis = wp.tile([P, 1], I32, tag="cidp")
            nc.sync.dma_start(out=cis[:], in_=cidp[:])
            tks = wp.tile([P, NT], mybir.dt.int16, tag="tokid")
            nc.sync.dma_start(out=tks[:], in_=tokid[:])
            dms = wp.tile([P, NT], F32, tag="dmpr")
            nc.sync.dma_start(out=dms[:], in_=dmpr[:])

            # ================= Phase A: local router logits =================
            with (
                tc.tile_pool(name="rpool", bufs=1) as rp,
                tc.tile_pool(name="rlg", bufs=3) as rlg,
                tc.tile_pool(name="rps", bufs=2, space="PSUM") as rps,
            ):
                wrs = rp.tile([P, 64], F32, tag="wr")
                nc.sync.dma_start(out=wrs[:], in_=wr[:])
                brs = rp.tile([1, E], F32, tag="brr")
                nc.sync.dma_start(out=brs[:], in_=brr[:])
                xts = []
                for k in range(C // P):
                    t = rp.tile([P, 1024], F32, tag=f"xt_{k}")
                    nc.sync.dma_start(out=t[:], in_=xt[k * P:(k + 1) * P, :])
                    xts.append(t)
                for m in range(1024 // P):
                    ps = rps.tile([P, E], F32, tag="lg")
                    for k in range(C // P):
                        nc.tensor.matmul(
                            ps[:], lhsT=xts[k][:, m * P:(m + 1) * P],
                            rhs=wrs[:, k * E:(k + 1) * E],
                            start=(k == 0), stop=False)
                    nc.tensor.matmul(ps[:], lhsT=on1[:], rhs=brs[:],
                                     start=False, stop=True)
                    lgs = rlg.tile([P, E], F32, tag="lgout")
                    nc.scalar.activation(lgs[:], ps[:], AF.Copy)
                    nc.sync.dma_start(out=lgin[m * P:(m + 1) * P, :], in_=lgs[:])

            nc.gpsimd.collective_compute(
                "AllGather", OP.bypass, ins=[lgin[:]], outs=[lgout[:]],
                replica_groups=rg)
            if debug:
                nc.sync.dma_start(out=dbg_lg[:], in_=lgout[:])
            # zero-fill buf shards (scatter-add targets) + agout pad rows;
            # scalar-queue DMAs, behind the router in priority
            zi = wp.tile([P, P], mybir.dt.int16, tag="zi")
            nc.vector.memset(zi[:], 0)
            for b in buf_ds:
                for i in range((CAP + P) // P):
                    nc.scalar.dma_start(out=b[i * P:(i + 1) * P, :], in_=zi[:])
            zb = wp.tile([P, C], BF16, tag="zb")
            nc.vector.memset(zb[:], 0.0)
            nc.scalar.dma_start(out=agout[E * CAP:E * CAP + P, :], in_=zb[:])

            # ================= Phase B: full routing ========================
            with (
                tc.tile_pool(name="bpool", bufs=1) as bp,
                tc.tile_pool(name="bps", bufs=2, space="PSUM") as bps,
            ):
                W = NT * E  # 512
                # token-id rows for the buf scatter (256B-replicated) — DVE
                # builds this while waiting for the logits AllGather
                tkb = bp.tile([P, NT * P], mybir.dt.int16, tag="tkb")
                nc.vector.tensor_copy(
                    out=tkb[:].rearrange("p (n e) -> p n e", e=P),
                    in_=tks[:].to_broadcast([P, NT, P]))
                L = bp.tile([P, W], F32, tag="L")
                nc.sync.dma_start(
                    out=_r3(L[:]),
                    in_=lgout[:].rearrange("(p n) e -> p n e", p=P))
                bv = bp.tile([P, W], F32, tag="binv")
                nc.sync.dma_start(out=bv[:], in_=binv[:])
                cm = bp.tile([P, W], F32, tag="cmask")
                nc.sync.dma_start(out=cm[:], in_=cmask[:])
                trs = bp.tile([P, P], F32, tag="tri")
                nc.sync.dma_start(out=trs[:], in_=tri[:])

                def tt(o, a, b, op):
                    nc.vector.tensor_tensor(out=o, in0=a, in1=b, op=op)

                v1 = bp.tile([P, NT], F32, tag="v1")
                nc.vector.reduce_max(v1[:], _r3(L[:]), axis=AX.X)
                m1 = bp.tile([P, W], F32, tag="t0")
                tt(_r3(m1[:]), _r3(L[:]), v1[:].to_broadcast([P, NT, E]),
                   OP.is_equal)
                tmp = bp.tile([P, W], F32, tag="t1")
                tt(tmp[:], m1[:], bv[:], OP.mult)
                e1x = bp.tile([P, NT], F32, tag="e1x")
                nc.vector.reduce_max(e1x[:], _r3(tmp[:]), axis=AX.X)
                oh1 = bp.tile([P, W], F32, tag="oh1")
                tt(_r3(oh1[:]), _r3(bv[:]), e1x[:].to_broadcast([P, NT, E]),
                   OP.is_equal)
                msk = bp.tile([P, W], F32, tag="t0b")
                nc.vector.tensor_scalar(out=msk[:], in0=oh1[:], scalar1=-BIG,
                                        scalar2=None, op0=OP.mult)
                tt(msk[:], L[:], msk[:], OP.add)
                v2 = bp.tile([P, NT], F32, tag="v2")
                nc.vector.reduce_max(v2[:], _r3(msk[:]), axis=AX.X)
                m2 = bp.tile([P, W], F32, tag="t2")
                tt(_r3(m2[:]), _r3(msk[:]), v2[:].to_broadcast([P, NT, E]),
                   OP.is_equal)
                tt(m2[:], m2[:], bv[:], OP.mult)
                e2x = bp.tile([P, NT], F32, tag="e2x")
                nc.vector.reduce_max(e2x[:], _r3(m2[:]), axis=AX.X)
                oh2 = bp.tile([P, W], F32, tag="oh2")
                tt(_r3(oh2[:]), _r3(bv[:]), e2x[:].to_broadcast([P, NT, E]),
                   OP.is_equal)
                sel = bp.tile([P, W], F32, tag="sel")
                tt(sel[:], oh1[:], oh2[:], OP.add)

                vd = bp.tile([P, NT], F32, tag="vd")
                tt(vd[:], v1[:], v2[:], OP.subtract)
                g1 = bp.tile([P, NT], F32, tag="g1")
                nc.scalar.activation(g1[:], vd[:], AF.Sigmoid)
                g2 = bp.tile([P, NT], F32, tag="g2")
                nc.vector.tensor_scalar(out=g2[:], in0=g1[:], scalar1=-1.0,
                                        scalar2=1.0, op0=OP.mult, op1=OP.add)

                # inclusive prefix over n (shift s tokens == 8s columns)
                cur = sel
                pidx = 0
                for s in (1, 2, 4, 8, 16, 32):
                    nxt = bp.tile([P, W], F32, tag=f"pf{pidx % 2}")
                    pidx += 1
                    tt(nxt[:, 8 * s:], cur[:, 8 * s:], cur[:, :W - 8 * s], OP.add)
                    nc.vector.tensor_copy(out=nxt[:, :8 * s], in_=cur[:, :8 * s])
                    cur = nxt
                incl = cur
                offp = bps.tile([P, E], F32, tag="offp")
                nc.tensor.matmul(offp[:], lhsT=trs[:], rhs=incl[:, W - E:W],
                                 start=True, stop=True)
                offs = bp.tile([P, E], F32, tag="offs")
                nc.scalar.activation(offs[:], offp[:], AF.Copy)
                rank = bp.tile([P, W], F32, tag="rank")
                tt(rank[:], incl[:], sel[:], OP.subtract)
                offs3 = bass.AP(
                    offs[:].tensor, offs[:].offset,
                    [offs[:].ap[0], [0, NT], offs[:].ap[1]])
                tt(_r3(rank[:]), _r3(rank[:]), offs3, OP.add)

                # dispatch indices for my expert: disp = rank + (1-sel)*BIG,
                # clamped to the dump row CAP, as int16 token-major
                disp = bp.tile([P, W], F32, tag="disp")
                nc.vector.tensor_scalar(out=disp[:], in0=sel[:], scalar1=-BIG,
                                        scalar2=BIG, op0=OP.mult, op1=OP.add)
                tt(disp[:], rank[:], disp[:], OP.add)
                dcf = bp.tile([P, W], F32, tag="dcf")
                tt(dcf[:], disp[:], cm[:], OP.mult)
                dce = bp.tile([P, NT], F32, tag="dce")
                nc.vector.reduce_sum(dce[:], _r3(dcf[:]), axis=AX.X)
                # clamp dropped entries onto per-token-spread dump rows so the
                # scatter-add's CCE RMW chains don't hammer one HBM row
                dcc = bp.tile([P, NT], F32, tag="dcc")
                tt(dcc[:], dce[:], dms[:], OP.min)
                dci = bp.tile([P, NT], mybir.dt.int16, tag="dci")
                nc.vector.tensor_copy(out=dci[:], in_=dcc[:])
                # bounce token-major to DRAM, reload in the 16-partition
                # index-wrap layout the batched SWDGE ops expect, replicate to
                # all 8 Q7 core blocks
                nc.sync.dma_start(
                    out=didx_d[:].rearrange("(p n) -> p n", p=P), in_=dci[:])
                didx = wp.tile([P, W], mybir.dt.int16, tag="didx")
                nc.sync.dma_start(
                    out=didx[0:16, :].rearrange("q (n b) -> q n b", b=8),
                    in_=didx_d[:].rearrange("(b q n) -> q n b", q=16, n=NT))
                for r in range(1, 8):
                    nc.sync.dma_start(out=didx[16 * r:16 * (r + 1), :],
                                      in_=didx[0:16, :])

                # combine indices, chunk-strided AG layout:
                # idx = (r>>8)*2048 + e*256 + (r&255)  (+BIG if dropped/unsel)
                def mkidx(ohx, exx, tag):
                    ei = bp.tile([P, NT], F32, tag=f"ei{tag}")
                    nc.vector.tensor_scalar(out=ei[:], in0=exx[:], scalar1=-1.0,
                                            scalar2=BIG, op0=OP.mult, op1=OP.add)
                    tmpr = bp.tile([P, W], F32, tag="t3")
                    tt(tmpr[:], rank[:], ohx[:], OP.mult)
                    ri = bp.tile([P, NT], F32, tag=f"ri{tag}")
                    nc.vector.reduce_sum(ri[:], _r3(tmpr[:]), axis=AX.X)
                    rii = bp.tile([P, NT], I32, tag=f"rii{tag}")
                    nc.vector.tensor_copy(out=rii[:], in_=ri[:])
                    rc = bp.tile([P, NT], I32, tag=f"rc{tag}")
                    nc.vector.tensor_scalar(out=rc[:], in0=rii[:], scalar1=8,
                                            scalar2=11, op0=OP.arith_shift_right,
                                            op1=OP.logical_shift_left)
                    rl = bp.tile([P, NT], I32, tag=f"rl{tag}")
                    nc.vector.tensor_scalar(out=rl[:], in0=rii[:], scalar1=255,
                                            scalar2=None, op0=OP.bitwise_and)
                    tt(rc[:], rc[:], rl[:], OP.add)
                    # dropped: r >= CAP -> +BIG
                    di = bp.tile([P, NT], F32, tag=f"di{tag}")
                    nc.vector.tensor_scalar(out=di[:], in0=ri[:],
                                            scalar1=float(CAP) - 0.5,
                                            scalar2=BIG,
                                            op0=OP.is_gt, op1=OP.mult)
                    ix = bp.tile([P, NT], F32, tag=f"ix{tag}")
                    nc.vector.tensor_scalar(out=ix[:], in0=ei[:],
                                            scalar1=256.0, scalar2=None,
                                            op0=OP.mult)
                    tt(ix[:], ix[:], di[:], OP.add)
                    rcf = bp.tile([P, NT], F32, tag=f"rcf{tag}")
                    nc.vector.tensor_copy(out=rcf[:], in_=rc[:])
                    tt(ix[:], ix[:], rcf[:], OP.add)
                    # dropped contributions point at the zeroed pad row
                    nc.vector.tensor_scalar(out=ix[:], in0=ix[:],
                                            scalar1=float(E * CAP),
                                            scalar2=None, op0=OP.min)
                    return ix

                ix1 = mkidx(oh1, e1x, "1")
                ix2 = mkidx(oh2, e2x, "2")

                pk = bp.tile([P, NT * 4], F32, tag="pk")
                pk4 = pk[:].rearrange("p (n f) -> p n f", f=4)
                nc.vector.tensor_copy(out=pk4[:, :, 0], in_=ix1[:])
                nc.vector.tensor_copy(out=pk4[:, :, 1], in_=ix2[:])
                nc.vector.tensor_copy(out=pk4[:, :, 2], in_=g1[:])
                nc.vector.tensor_copy(out=pk4[:, :, 3], in_=g2[:])
                nc.sync.dma_start(
                    out=rsc[:].rearrange("(p q) f -> p q f", p=P), in_=pk4)
                if debug:
                    nc.sync.dma_start(out=dbg_rs[:], in_=rsc[:])

                # ============== Phase C: dispatch scatter ===================
                # own-token combine metadata prefetch (independent of AGs)
                own = wp.tile([P, 32], F32, tag="own")
                nc.gpsimd.indirect_dma_start(
                    out=own[:], out_offset=None,
                    in_=rsc[:],
                    in_offset=bass.IndirectOffsetOnAxis(ap=cis[:, :1], axis=0),
                )
                own4 = own[:].rearrange("p (n f) -> p n f", f=4)
                i1 = wp.tile([P, 8], mybir.dt.int16, tag="i1")
                nc.vector.tensor_copy(out=i1[:], in_=own4[:, :, 0])
                i2 = wp.tile([P, 8], mybir.dt.int16, tag="i2")
                nc.vector.tensor_copy(out=i2[:], in_=own4[:, :, 1])
                go1 = wp.tile([P, 8], F32, tag="go1")
                nc.vector.tensor_copy(out=go1[:], in_=own4[:, :, 2])
                go2 = wp.tile([P, 8], F32, tag="go2")
                nc.vector.tensor_copy(out=go2[:], in_=own4[:, :, 3])
                # combine-gather index tiles: entry i = p' + 128*n' for own
                # token j = p'*8 + n', in 16-partition wrap, replicated
                cidx = []
                for f, it in ((0, i1), (1, i2)):
                    nc.sync.dma_start(
                        out=cscr[f, :].rearrange("(p n) -> p n", p=P), in_=it[:])
                    ci = wp.tile([P, 64], mybir.dt.int16, tag=f"cidx{f}")
                    nc.sync.dma_start(
                        out=ci[0:16, :].rearrange("q (n b) -> q n b", b=8),
                        in_=cscr[f, :].rearrange("(b q n) -> q n b", q=16, n=8))
                    for r in range(1, 8):
                        nc.sync.dma_start(out=ci[16 * r:16 * (r + 1), :],
                                          in_=ci[0:16, :])
                    cidx.append(ci)

                tkb3 = tkb[:].rearrange("p (n e) -> p n e", e=P)
                for k in range(NTOK // 512):
                    nc.gpsimd.dma_scatter_add(
                        out_ap=buf_ds[k % 4][:],
                        in_ap=tkb3[:, k * 4:(k + 1) * 4, :],
                        idxs_ap=didx[:, k * 32:(k + 1) * 32],
                        num_idxs=512,
                        num_idxs_reg=512,
                        elem_size=P,
                        single_packet=False,
                    )
                # reload the four buf shards (each slot written by exactly one
                # scatter chain; the rest hold zeros) and sum them
                bufi = wp.tile([P, CAP // 16], mybir.dt.int16, tag="bufi")
                parts = []
                for i in range(4):
                    t = wp.tile([16, CAP // 16], mybir.dt.int16, tag=f"bfp{i}")
                    nc.sync.dma_start(
                        out=t[:],
                        in_=buf_ds[i][0:CAP, 0:1].rearrange(
                            "(s q) one -> q (s one)", q=16))
                    parts.append(t)
                nc.vector.tensor_tensor(out=parts[0][:], in0=parts[0][:],
                                        in1=parts[1][:], op=OP.add)
                nc.vector.tensor_tensor(out=parts[2][:], in0=parts[2][:],
                                        in1=parts[3][:], op=OP.add)
                nc.vector.tensor_tensor(out=bufi[0:16, :], in0=parts[0][:],
                                        in1=parts[2][:], op=OP.add)
                for r in range(1, 8):
                    nc.sync.dma_start(out=bufi[16 * r:16 * (r + 1), :],
                                      in_=bufi[0:16, :])

            # ---- weights (loaded behind router/dispatch in priority order) --
            w1t = []
            for k in range(C // P):
                t = wp.tile([P, HD], BF16, tag=f"w1_{k}")
                nc.sync.dma_start(out=t[:], in_=w1[k * P:(k + 1) * P, :])
                w1t.append(t)
            w2t = []
            for k in range(HD // P):
                t = wp.tile([P, C], BF16, tag=f"w2_{k}")
                nc.sync.dma_start(out=t[:], in_=w2[k * P:(k + 1) * P, :])
                w2t.append(t)
            b1s = wp.tile([P, HD // P], F32, tag="b1a")
            nc.sync.dma_start(out=b1s[:], in_=b1a[:])
            b2s = wp.tile([1, C], F32, tag="b2r")
            nc.sync.dma_start(out=b2s[:], in_=b2r[:])

            # ================= Phase D: expert FFN ==========================
            with (
                tc.tile_pool(name="fpool", bufs=1) as fp,
                tc.tile_pool(name="fps", bufs=4, space="PSUM") as fpsH,
                tc.tile_pool(name="fps2", bufs=2, space="PSUM") as fpsM,
                tc.tile_pool(name="xgt", bufs=2) as xp,
                tc.tile_pool(name="osb", bufs=3) as op_,
            ):
                for chk in range(NCH):
                    gout = xp.tile([P, (TCH // P) * C], BF16, tag="gout")
                    gout3 = gout[:].rearrange("p (a c) -> p a c", c=C)
                    nc.gpsimd.dma_gather(
                        out_ap=gout3,
                        in_ap=xb[:],
                        idxs_ap=bufi[:, chk * (TCH // 16):(chk + 1) * (TCH // 16)],
                        num_idxs=TCH,
                        num_idxs_reg=TCH,
                        elem_size=C,
                        transpose=False,
                        single_packet=False,
                    )
                    nc.sync.dma_start(
                        out=xg[chk * TCH:(chk + 1) * TCH, :].rearrange(
                            "(a p) c -> p a c", p=P),
                        in_=gout3)
                    xgt = xp.tile([P, (C // P) * TCH], BF16, tag="xgt")
                    xgt3 = xgt[:].rearrange("p (k s) -> p k s", s=TCH)
                    for k in range(C // P):
                        nc.scalar.dma_start(
                            out=xgt3[:, k, :],
                            in_=xg[chk * TCH:(chk + 1) * TCH, k * P:(k + 1) * P],
                            transpose=True)
                    hs = []
                    for m in range(HD // P):
                        ps = fpsH.tile([P, TCH], F32, tag="hps")
                        for k in range(C // P):
                            nc.tensor.matmul(
                                ps[:], lhsT=w1t[k][:, m * P:(m + 1) * P],
                                rhs=xgt3[:, k, :], start=(k == 0),
                                stop=(k == C // P - 1))
                        h = fp.tile([P, TCH], BF16, tag=f"h_{chk % 2}_{m}")
                        nc.scalar.activation(h[:], ps[:], AF.Gelu,
                                             bias=b1s[:, m:m + 1])
                        hs.append(h)
                    for st in range(TCH // P):
                        for cf in range(2):
                            ps2 = fpsM.tile([P, 512], F32, tag="ops")
                            for k in range(HD // P):
                                nc.tensor.matmul(
                                    ps2[:], lhsT=hs[k][:, st * P:(st + 1) * P],
                                    rhs=w2t[k][:, cf * 512:(cf + 1) * 512],
                                    start=(k == 0), stop=False)
                            nc.tensor.matmul(
                                ps2[:], lhsT=on1[:],
                                rhs=b2s[:, cf * 512:(cf + 1) * 512],
                                start=False, stop=True)
                            ob = op_.tile([P, 512], BF16, tag="ob")
                            nc.scalar.activation(ob[:], ps2[:], AF.Copy)
                            nc.sync.dma_start(
                                out=agin[chk * TCH + st * P:
                                         chk * TCH + (st + 1) * P,
                                         cf * 512:(cf + 1) * 512],
                                in_=ob[:])
                    # ---- chunked output AllGather (overlaps later chunks) ---
                    if (chk + 1) % (AGCH // TCH) == 0:
                        ag = (chk + 1) // (AGCH // TCH) - 1
                        nc.gpsimd.collective_compute(
                            "AllGather", OP.bypass,
                            ins=[agin[ag * AGCH:(ag + 1) * AGCH, :]],
                            outs=[agout[ag * E * AGCH:(ag + 1) * E * AGCH, :]],
                            replica_groups=rg)

            if debug:
                dbx = wp.tile([P, C], F32, tag="dbx")
                dbb = wp.tile([P, P], mybir.dt.int16, tag="dbb")
                dbf = wp.tile([P, P], F32, tag="dbf")
                for i in range((CAP + P) // P):
                    nc.sync.dma_start(out=dbb[:], in_=buf_d[i * P:(i + 1) * P, :])
                    nc.vector.tensor_copy(out=dbf[:], in_=dbb[:])
                    nc.sync.dma_start(out=dbg_bf[i * P:(i + 1) * P, :], in_=dbf[:])
                for i in range(E * CAP // P):
                    nc.sync.dma_start(out=dbx[:], in_=agout[i * P:(i + 1) * P, :])
                    nc.sync.dma_start(out=dbg_ag[i * P:(i + 1) * P, :], in_=dbx[:])

            # ================= Phase F: combine own tokens ==================
            with tc.tile_pool(name="cpool", bufs=1) as cp:
                cg = []
                for f in range(2):
                    halves = []
                    for h in range(2):
                        t = cp.tile([P, 4 * C], BF16, tag=f"cg{f}_{h}")
                        nc.gpsimd.dma_gather(
                            out_ap=t[:].rearrange("p (n c) -> p n c", c=C),
                            in_ap=agout[:],
                            idxs_ap=cidx[f][:, h * 32:(h + 1) * 32],
                            num_idxs=512,
                            num_idxs_reg=512,
                            elem_size=C,
                            transpose=False,
                            single_packet=False,
                        )
                        halves.append(t[:].rearrange("p (n c) -> p n c", c=C))
                    cg.append(halves)
                with tc.tile_pool(name="copool", bufs=3) as cop:
                    for j in range(8):
                        h, jj = j // 4, j % 4
                        o1 = cop.tile([P, C], F32, tag="o1")
                        nc.vector.tensor_scalar(out=o1[:], in0=cg[0][h][:, jj, :],
                                                scalar1=go1[:, j:j + 1],
                                                scalar2=None, op0=OP.mult)
                        o2 = cop.tile([P, C], F32, tag="o2")
                        nc.vector.tensor_scalar(out=o2[:], in0=cg[1][h][:, jj, :],
                                                scalar1=go2[:, j:j + 1],
                                                scalar2=None, op0=OP.mult)
                        nc.vector.tensor_tensor(out=o1[:], in0=o1[:], in1=o2[:],
                                                op=OP.add)
                        nc.sync.dma_start(
                            out=out[:].rearrange("(p n) c -> p n c", p=P)[:, j, :],
                            in_=o1[:])

    nc.compile()
    return nc


def _host_inputs(x, Wr, br, W1, b1, W2, b2):
    xf = np.ascontiguousarray(x.reshape(NTOK, C)).astype(np.float32)
    xb = xf.astype(ml_dtypes.bfloat16)
    wr_a = np.ascontiguousarray(
        Wr.reshape(C // P, P, E).transpose(1, 0, 2).reshape(P, 64)).astype(
            np.float32)
    brr = br.reshape(1, E).astype(np.float32)
    tri = np.triu(np.ones((P, P), np.float32), 1)
    binv = np.broadcast_to(
        np.tile(BIG - np.arange(E, dtype=np.float32), NT), (P, NT * E)).copy()
    ones1 = np.ones((1, P), np.float32)
    tokid = (np.arange(P, dtype=np.int32)[:, None] * NT
             + np.arange(NT, dtype=np.int32)[None, :]).astype(np.int16)
    dmpr = (CAP + (np.arange(P)[:, None] * NT + np.arange(NT)[None, :]) % P
            ).astype(np.float32)
    in_maps = []
    for c in range(NCORES):
        cm = np.zeros(E, np.float32)
        cm[c] = 1.0
        cmask = np.broadcast_to(np.tile(cm, NT), (P, NT * E)).copy()
        in_maps.append({
            "xt": np.ascontiguousarray(xf[c * 1024:(c + 1) * 1024, :].T),
            "xb": xb,
            "wr": wr_a,
            "brr": brr,
            "w1": np.ascontiguousarray(W1[c]).astype(ml_dtypes.bfloat16),
            "b1a": np.ascontiguousarray(
                b1[c].reshape(HD // P, P).T).astype(np.float32),
            "w2": np.ascontiguousarray(W2[c]).astype(ml_dtypes.bfloat16),
            "b2r": b2[c].reshape(1, C).astype(np.float32),
            "tri": tri,
            "binv": binv,
            "cmask": cmask,
            "ones1": ones1,
            "cidp": (c * P + np.arange(P, dtype=np.int32)).reshape(P, 1),
            "tokid": tokid,
            "dmpr": dmpr,
        })
    return in_maps


def _ensure_ntff_hook():
    """The agent image's antenv lacks axon_hooks; shim it so trace=True works."""
    import types
    try:
        import antenv.axon_hooks  # noqa: F401
        return
    except ImportError:
        pass
    import antenv
    mod = types.ModuleType("antenv.axon_hooks")
    state = {"h": None}
    mod.set_axon_ntff_profile_hook = lambda h: state.__setitem__("h", h)
    mod.get_axon_ntff_profile_hook = lambda: state["h"]
    sys.modules["antenv.axon_hooks"] = mod
    antenv.axon_hooks = mod
    from trn_agent_boot.trn_boot import _ntff_profile_via_ctypes
    mod.set_axon_ntff_profile_hook(
        _ntff_profile_via_ctypes("/opt/axon/libaxon_pjrt.so"))


def kernel(x, Wr, br, W1, b1, W2, b2, _debug=False, _trace=False):
    global _BUILT
    x, Wr, br = np.asarray(x), np.asarray(Wr), np.asarray(br)
    W1, b1, W2, b2 = map(np.asarray, (W1, b1, W2, b2))
    if _BUILT is None or _BUILT[1] != _debug:
        _BUILT = (build(debug=_debug), _debug)
    nc = _BUILT[0]
    in_maps = _host_inputs(x, Wr, br, W1, b1, W2, b2)
    if _trace:
        _ensure_ntff_hook()
    res = run_bass_kernel_spmd(nc, in_maps, list(range(NCORES)), trace=_trace)
    outs = np.concatenate([res.results[c]["out"] for c in range(NCORES)], 0)
    out = outs.reshape(x.shape).astype(np.float32)
    if _debug:
        kernel.debug_results = res
    if _trace:
        kernel.trace_results = res
    return out
